# revision 39
# baseline (speedup 1.0000x reference)
"""TRN2 Bass kernel for nn_GCNBasic (2-layer GCN, B=32, N=2048, F=128, H=256).

Sharding: data-parallel over batch B across 8 NeuronCores (4 items/core);
small weights replicated.  A_hat is scaled by 2^17, cast to fp8-e4m3 and
transposed on the HOST (quarters HBM traffic vs f32); X and H1 are also
fp8, so BOTH aggregation matmuls (88% of the FLOPs) run in DoubleRow fp8
perf mode: each PE instruction contracts TWO 128-deep k-planes
(stationary [128,2,128] fp8, moving [128,2,512] fp8) at 2x bf16 MACs
(measured 212 ns per [128,512]-out instruction, LDWEIGHTS fully
overlapped with the previous matmul's stream).  The 2^-17 unscale is
folded into W1/W2 (bf16 holds tiny values exactly), so all LayerNorm
math is numerically identical to a bf16 kernel.  W2 stays bf16: an fp8
W2's quantization error is constant across nodes and survives the
mean-pool readout (measured 2.6e-2 final error vs 4.0e-3 with W2 bf16).

  (AX)^T[f,n]  = sum_c2 X[pair]-stationary  @ A^T[pair]   (fp8 DoubleRow,
                                              4 psum 512-chunks live)
  H1pre[n,h]   = (AX)^T[:,nb]-stat. @ W1/2^17 (bf16)
  H1           = relu(LN(H1pre + b1)) -> fp8  (bias-add+rowsum in one
                 DVE scalar_tensor_tensor with accum_out — this is also
                 the PSUM drain, so PSUM recycles after ~420ns; sumsq in
                 a second DVE stt; relu apply on ACT from SBUF staging)
  (AH)^T[hh,n] = sum_c2 H1[pair]-stationary  @ A^T[pair]   (fp8 DoubleRow)
  H2pre[n,k]   = sum_hh (AH)^T[hh,nb]-stat.  @ diag(g1)W2/2^17
  H2           = relu(LN(H2pre + b2)) -> bf16
  g            = 1^T H2 / N  (16 ones-STATIONARY matmuls into a [1,256]
                 PSUM row — weight loads are trivial — then 2 tiny PE
                 transposes give g^T halves for the f32 heads)

Items are software-pipelined at dense-matmul granularity: each dense
phase's 16 block-matmuls (+ their LN stats ops) drip into the NEXT
aggregation phase's c2 loop; finish_stats + applies run in quarters of
4 blocks so early applies overlap later stats; pool/head blocks trail
their LN2 applies by one phase (pool PSUM borrows an idle ps_big bank);
the last item fuses its mean-pool matmuls into the apply quarters.
A^T for item it+2 is prefetched a full phase ahead (pool_at holds 3
items, 12 MiB) so its ~11us HBM transfer never exposes.

gamma folds (diag(g1)@W2, diag(g2)@Wa/Wl) are exact because relu(g*z)=
g*relu(z) for g>0; beta==0 fast path (the problem's setup_inputs always
produces gamma=1, beta=0); a general gamma/beta path exists as a fallback.

Known TRN2 pitfalls worked around here: tensor_tensor_reduce crashes the
device; ACT/DVE writes into PSUM are unstable (reads are fine); Pool
(gpsimd) has no PSUM access and no TensorScalarPtr opcode; PSUM-resident
pre-activations gate the in-order PE queue on the slow apply chain, so
pre-activations are drained to SBUF by the bias-add stt instead.
"""

from contextlib import ExitStack

import numpy as np
import ml_dtypes

import concourse.bacc as bacc
import concourse.mybir as mybir
import concourse.tile as tile
from concourse.bass_utils import run_bass_kernel_spmd

F32 = mybir.dt.float32
BF16 = mybir.dt.bfloat16
FP16 = mybir.dt.float16
F8 = mybir.dt.float8e4
bf16 = ml_dtypes.bfloat16
f8e4 = ml_dtypes.float8_e4m3

N = 2048
F = 128
H = 256
K = 64
P = 128
NB = N // P
NB2 = NB // 2
NCH = N // 512
EPS = 1e-5
N_CORES = 8
ASCALE = 2.0 ** 17  # A_hat -> e4m3 range; 1/ASCALE folded into W1/W2
DR = mybir.MatmulPerfMode.DoubleRow


def _declare_io(nc, items, general):
    io = {}
    io["at4"] = nc.dram_tensor("at4", [items, NB2, P, 2, N], F8,
                               kind="ExternalInput")
    io["x4"] = nc.dram_tensor("x4", [items, P, NB, F], F8,
                              kind="ExternalInput")
    io["w1"] = nc.dram_tensor("w1", [F, H], BF16, kind="ExternalInput")
    io["w2"] = nc.dram_tensor("w2", [H, H], BF16, kind="ExternalInput")
    io["b1bc"] = nc.dram_tensor("b1bc", [P, H], F32, kind="ExternalInput")
    io["b2bc"] = nc.dram_tensor("b2bc", [P, H], F32, kind="ExternalInput")
    io["wa"] = nc.dram_tensor("wa", [H, K], F32, kind="ExternalInput")
    io["wl"] = nc.dram_tensor("wl", [H, K], F32, kind="ExternalInput")
    io["ba"] = nc.dram_tensor("ba", [K, 1], F32, kind="ExternalInput")
    io["bl"] = nc.dram_tensor("bl", [K, 1], F32, kind="ExternalInput")
    io["ones"] = nc.dram_tensor("ones", [P, 1], BF16, kind="ExternalInput")
    if general:
        io["g1bc"] = nc.dram_tensor("g1bc", [P, H], F32, kind="ExternalInput")
        io["be1bc"] = nc.dram_tensor("be1bc", [P, H], F32,
                                     kind="ExternalInput")
        io["g2bc"] = nc.dram_tensor("g2bc", [P, H], F32, kind="ExternalInput")
        io["be2bc"] = nc.dram_tensor("be2bc", [P, H], F32,
                                     kind="ExternalInput")
    io["op"] = nc.dram_tensor("op", [items, K], F32, kind="ExternalOutput")
    io["ol"] = nc.dram_tensor("ol", [items, K], F32, kind="ExternalOutput")
    return io


def _build_core(nc, tc, io, items, general):
    at4, x4 = io["at4"], io["x4"]
    es = ExitStack()

    consts = es.enter_context(tc.tile_pool(name="consts", bufs=1))
    wts = es.enter_context(tc.tile_pool(name="wts", bufs=1))
    pool_at = es.enter_context(tc.tile_pool(name="at", bufs=2 * NB2))
    pool_xb = es.enter_context(tc.tile_pool(name="xb", bufs=2))
    pool_axT = es.enter_context(tc.tile_pool(name="axT", bufs=2))
    pool_h1 = es.enter_context(tc.tile_pool(name="h1", bufs=2))
    pool_ahT = es.enter_context(tc.tile_pool(name="ahT", bufs=2))
    pool_h2 = es.enter_context(tc.tile_pool(name="h2", bufs=1))
    pool_hc = es.enter_context(tc.tile_pool(name="hc", bufs=NB))
    pool_sq = es.enter_context(tc.tile_pool(name="sq", bufs=2))
    pool_st = es.enter_context(tc.tile_pool(name="st", bufs=4))
    pool_gsb = es.enter_context(tc.tile_pool(name="gsb", bufs=4))
    pool_osb = es.enter_context(tc.tile_pool(name="osb", bufs=4))

    ps_big = es.enter_context(tc.tile_pool(name="ps_big", bufs=5, space="PSUM"))
    ps_h = es.enter_context(tc.tile_pool(name="ps_h", bufs=3, space="PSUM"))
    ps_sm = ps_h  # pool/head tiles share the dense-phase banks

    eps_t = consts.tile([P, 1], F32)
    nc.vector.memset(eps_t[:], EPS)
    id1 = consts.tile([1, 1], FP16)
    nc.vector.memset(id1[:], 1.0)
    ones_b = consts.tile([P, 1], BF16)
    w1_t = wts.tile([P, H], BF16)
    w2_t = [wts.tile([P, H], BF16, tag=f"w2_{hh}", name=f"w2_{hh}")
            for hh in range(2)]
    b1_t = wts.tile([P, H], F32)
    b2_t = wts.tile([P, H], F32)
    wa_t = [wts.tile([P, K], F32, tag=f"wa_{hh}", name=f"wa_{hh}")
            for hh in range(2)]
    wl_t = [wts.tile([P, K], F32, tag=f"wl_{hh}", name=f"wl_{hh}")
            for hh in range(2)]
    ba_t = wts.tile([K, 1], F32)
    bl_t = wts.tile([K, 1], F32)
    gb_t = {}
    if general:
        for nm in ("g1bc", "be1bc", "g2bc", "be2bc"):
            gb_t[nm] = wts.tile([P, H], F32, tag=nm, name=nm)

    def emit_weight_dmas():
        nc.sync.dma_start(ones_b[:], io["ones"][:])
        nc.sync.dma_start(w1_t[:], io["w1"][:])
        for hh in range(2):
            nc.sync.dma_start(w2_t[hh][:], io["w2"][hh * P:(hh + 1) * P, :])
        nc.sync.dma_start(b1_t[:], io["b1bc"][:])
        nc.sync.dma_start(b2_t[:], io["b2bc"][:])
        for hh in range(2):
            nc.sync.dma_start(wa_t[hh][:], io["wa"][hh * P:(hh + 1) * P, :])
            nc.sync.dma_start(wl_t[hh][:], io["wl"][hh * P:(hh + 1) * P, :])
        nc.sync.dma_start(ba_t[:], io["ba"][:])
        nc.sync.dma_start(bl_t[:], io["bl"][:])
        for nm, t in gb_t.items():
            nc.sync.dma_start(t[:], io[nm][:])

    inv_h = 1.0 / H

    # per-item live tiles (indexed by item)
    at_t = [None] * items
    xb_t = [None] * items
    axT_t = [None] * items
    h1_t = [None] * items
    ahT_t = [None] * items
    h2_t = [None] * items
    st1_t = [None] * items
    st2_t = [None] * items

    def load(it, chunks=1):
        xb = pool_xb.tile([P, NB, F], F8, tag="xb", name=f"xb_{it}")
        nc.sync.dma_start(xb[:], x4[it])
        xb_t[it] = xb
        ats = [pool_at.tile([P, 2, N], F8, tag="at", name=f"at_{it}_{c2}")
               for c2 in range(NB2)]
        cw = N // chunks
        for c2 in range(NB2):
            for k in range(chunks):
                nc.sync.dma_start(
                    ats[c2][:, :, k * cw:(k + 1) * cw],
                    at4[it, c2, :, :, k * cw:(k + 1) * cw])
        at_t[it] = ats

    def copy_out(dst, src, who):
        # PSUM->SBUF copy-outs alternate ACT/DVE (Pool cannot read PSUM)
        if who % 2 == 0:
            nc.scalar.copy(dst, src)
        else:
            nc.vector.tensor_copy(dst, src)

    def l1_agg(it, unit=None):
        # two pending dense units (PE matmul + LN stats) dripped per c2
        # iteration so the LN chain paces alongside pure agg matmuls
        at, xb = at_t[it], xb_t[it]
        pb = [ps_big.tile([P, 512], F32, tag="big", name=f"ax_{it}_{j}")
              for j in range(NCH)]
        for c2 in range(NB2):
            for j in range(NCH):
                nc.tensor.matmul(pb[j][:], xb[:, 2 * c2:2 * c2 + 2, :],
                                 at[c2][:, :, j * 512:(j + 1) * 512],
                                 start=(c2 == 0), stop=(c2 == NB2 - 1),
                                 perf_mode=DR)
            if unit is not None:
                unit(2 * c2)
                unit(2 * c2 + 1)
        axT = pool_axT.tile([P, N], BF16, tag="axT", name=f"axT_{it}")
        for j in range(NCH):
            copy_out(axT[:, j * 512:(j + 1) * 512], pb[j][:], j)
        axT_t[it] = axT

    def ln_stats(nb, ph, b_t, st, hc, sfx):
        # bias add + LN row-sum in one DVE op (accum_out; this is also
        # the PSUM drain so the in-order PE queue is only gated ~420ns);
        # sum column H comes from the matmul's extra W column; sumsq in
        # a second DVE stt; ACT stays free for the applies
        nc.vector.scalar_tensor_tensor(
            out=hc[:], in0=ph[:, 0:H], scalar=1.0, in1=b_t[:],
            op0=mybir.AluOpType.mult, op1=mybir.AluOpType.add,
            accum_out=st[:, 0, nb:nb + 1])
        sq = pool_sq.tile([P, H], F32, tag="sq", name=f"sq_{sfx}")
        nc.vector.scalar_tensor_tensor(
            out=sq[:], in0=hc[:], scalar=1.0, in1=hc[:],
            op0=mybir.AluOpType.mult, op1=mybir.AluOpType.mult,
            accum_out=st[:, 1, nb:nb + 1])

    def finish_stats(st, lo=0, hi=NB):
        # runs on a [lo:hi) slice of the nb columns so early blocks'
        # applies overlap later blocks' stats (subtile deps)
        s = st[:, :, lo:hi]
        nc.vector.tensor_scalar(out=s[:, 2, :], in0=s[:, 0, :],
                                scalar1=-inv_h, scalar2=None,
                                op0=mybir.AluOpType.mult)          # -mu
        nc.vector.tensor_tensor(out=s[:, 3, :], in0=s[:, 2, :], in1=s[:, 2, :],
                                op=mybir.AluOpType.mult)           # mu^2
        nc.vector.tensor_scalar(out=s[:, 4, :], in0=s[:, 1, :],
                                scalar1=inv_h, scalar2=None,
                                op0=mybir.AluOpType.mult)          # E[x^2]
        nc.vector.tensor_tensor(out=s[:, 4, :], in0=s[:, 4, :], in1=s[:, 3, :],
                                op=mybir.AluOpType.subtract)       # var
        nc.scalar.activation(out=s[:, 5, :], in_=s[:, 4, :],
                             func=mybir.ActivationFunctionType.Sqrt,
                             bias=eps_t[:], scale=1.0)             # sd
        nc.vector.reciprocal(out=s[:, 6, :], in_=s[:, 5, :])       # 1/sd
        nc.vector.tensor_tensor(out=s[:, 7, :], in0=s[:, 2, :], in1=s[:, 6, :],
                                op=mybir.AluOpType.mult)           # -mu/sd

    def apply_ln(nb, hc, st, h_out, g_bc, be_bc):
        if not general:
            nc.scalar.activation(out=h_out, in_=hc[:],
                                 func=mybir.ActivationFunctionType.Relu,
                                 bias=st[:, 7, nb:nb + 1],
                                 scale=st[:, 6, nb:nb + 1])
        else:
            nc.scalar.activation(out=hc[:], in_=hc[:],
                                 func=mybir.ActivationFunctionType.Identity,
                                 bias=st[:, 7, nb:nb + 1],
                                 scale=st[:, 6, nb:nb + 1])
            nc.gpsimd.tensor_tensor(out=hc[:], in0=hc[:], in1=g_bc[:],
                                    op=mybir.AluOpType.mult)
            nc.vector.tensor_tensor(out=hc[:], in0=hc[:], in1=be_bc[:],
                                    op=mybir.AluOpType.add)
            nc.scalar.activation(out=h_out, in_=hc[:],
                                 func=mybir.ActivationFunctionType.Relu)

    def l2_agg(it, unit=None):
        at, h1 = at_t[it], h1_t[it]
        ahT = [pool_ahT.tile([P, N], BF16, tag="ahT", name=f"ahT_{it}_{hh}")
               for hh in range(2)]
        ucnt = 0
        for hh in range(2):
            pb = [ps_big.tile([P, 512], F32, tag="big",
                              name=f"ah_{it}_{hh}_{j}") for j in range(NCH)]
            for c2 in range(NB2):
                for j in range(NCH):
                    nc.tensor.matmul(
                        pb[j][:], h1[:, 2 * c2:2 * c2 + 2, hh * P:(hh + 1) * P],
                        at[c2][:, :, j * 512:(j + 1) * 512],
                        start=(c2 == 0), stop=(c2 == NB2 - 1), perf_mode=DR)
                if unit is not None:
                    unit(ucnt)
                    ucnt += 1
            for j in range(NCH):
                copy_out(ahT[hh][:, j * 512:(j + 1) * 512], pb[j][:],
                         j + hh)
        ahT_t[it] = ahT

    pg_t = [None] * items

    def make_l1_dense(it):
        axT = axT_t[it]
        st1 = pool_st.tile([P, 8, NB], F32, tag="st", name=f"st1_{it}")
        h1 = pool_h1.tile([P, NB, H], F8, tag="h1", name=f"h1_{it}")
        hc1 = []
        st1_t[it], h1_t[it] = st1, h1

        def unit(nb):
            ph = ps_h.tile([P, H], F32, tag="h", name=f"p1_{it}_{nb}")
            nc.tensor.matmul(ph[:], axT[:, nb * P:(nb + 1) * P], w1_t[:],
                             start=True, stop=True)
            hc = pool_hc.tile([P, H], F32, tag="hc", name=f"hc1_{it}_{nb}")
            ln_stats(nb, ph, b1_t, st1, hc, f"1_{it}_{nb}")
            hc1.append(hc)

        def fin():
            for q in range(0, NB, 4):
                finish_stats(st1, q, q + 4)
                for nb in range(q, q + 4):
                    apply_ln(nb, hc1[nb], st1, h1[:, nb, :],
                             gb_t.get("g1bc"), gb_t.get("be1bc"))

        return unit, fin

    def make_l2_dense(it, fuse_pool=False):
        ahT = ahT_t[it]
        st2 = pool_st.tile([P, 8, NB], F32, tag="st", name=f"st2_{it}")
        h2 = pool_h2.tile([P, NB, H], BF16, tag="h2", name=f"h2_{it}")
        hc2 = []
        st2_t[it], h2_t[it] = st2, h2

        def unit(nb):
            ph = ps_h.tile([P, H], F32, tag="h", name=f"p2_{it}_{nb}")
            for hh in range(2):
                nc.tensor.matmul(ph[:], ahT[hh][:, nb * P:(nb + 1) * P],
                                 w2_t[hh][:], start=(hh == 0), stop=(hh == 1))
            hc = pool_hc.tile([P, H], F32, tag="hc", name=f"hc2_{it}_{nb}")
            ln_stats(nb, ph, b2_t, st2, hc, f"2_{it}_{nb}")
            hc2.append(hc)

        def fin():
            if fuse_pool:
                # last item: mean-pool accumulation rides the apply
                # quarters so the tail chain stays short
                pg = [ps_sm.tile([P, 1], F32, tag="h", name=f"pg_{it}_{kh}")
                      for kh in range(2)]
                pg_t[it] = pg
            for q in range(0, NB, 4):
                finish_stats(st2, q, q + 4)
                for nb in range(q, q + 4):
                    apply_ln(nb, hc2[nb], st2, h2[:, nb, :],
                             gb_t.get("g2bc"), gb_t.get("be2bc"))
                    if fuse_pool:
                        for kh in range(2):
                            nc.tensor.matmul(pg[kh][:],
                                             h2[:, nb, kh * P:(kh + 1) * P],
                                             ones_b[:], start=(nb == 0),
                                             stop=(nb == NB - 1))

        return unit, fin

    def pool_block(it):
        h2 = h2_t[it]
        gsb = pool_gsb.tile([P, 2], F32, tag="g", name=f"g_{it}")
        if pg_t[it] is None:
            pg = [ps_sm.tile([P, 1], F32, tag="h", name=f"pg_{it}_{kh}")
                  for kh in range(2)]
            for nb in range(NB):
                for kh in range(2):
                    nc.tensor.matmul(pg[kh][:],
                                     h2[:, nb, kh * P:(kh + 1) * P],
                                     ones_b[:], start=(nb == 0),
                                     stop=(nb == NB - 1))
        else:
            pg = pg_t[it]
        for kh in range(2):
            nc.scalar.mul(gsb[:, kh:kh + 1], pg[kh][:], 1.0 / N)

        for hd, (w_t, b_t, out_d) in enumerate(
                ((wa_t, ba_t, io["op"]), (wl_t, bl_t, io["ol"]))):
            po = ps_sm.tile([K, 1], F32, tag="h", name=f"po_{it}_{hd}")
            for kh in range(2):
                nc.tensor.matmul(po[:], w_t[kh][:], gsb[:, kh:kh + 1],
                                 start=(kh == 0), stop=(kh == 1))
            osb = pool_osb.tile([K, 1], F32, tag="o", name=f"o_{it}_{hd}")
            nc.scalar.activation(out=osb[:], in_=po[:],
                                 func=mybir.ActivationFunctionType.Identity,
                                 bias=b_t[:], scale=1.0)
            nc.sync.dma_start(out_d[it:it + 1, :], osb[:])

    # ---- software pipeline: dense phases interleave into the next agg
    # phase's c2 loop; pool/head blocks trail their applies by one phase ----
    load(0, chunks=4)       # chunked so the first tiles land early
    emit_weight_dmas()
    if items == 1:
        l1_agg(0)
        unit, fin = make_l1_dense(0)
        for nb in range(NB):
            unit(nb)
        fin()
        l2_agg(0)
        unit, fin = make_l2_dense(0, fuse_pool=True)
        for nb in range(NB):
            unit(nb)
        fin()
        pool_block(0)
        es.close()
        return

    load(1, chunks=1)
    phases = [("l1", 0), ("l1", 1)]
    for it in range(items):
        phases.append(("l2", it))
        if it + 2 < items:
            phases.append(("l1", it + 2))

    ready = None            # (kind, it, unit, fin) pending dense phase
    pool_q = []             # items whose pool block is due next phase
    for kind, it in phases:
        pool_now, pool_q = pool_q, []
        cur, ready = ready, None
        unit = cur[2] if cur else None
        if kind == "l1":
            if it >= 2:
                load(it, chunks=1)
            l1_agg(it, unit)
        else:
            l2_agg(it, unit)
        if cur is not None:
            cur[3]()
            if cur[0] == "l2":
                pool_q.append(cur[1])
        for p in pool_now:
            pool_block(p)
        if kind == "l1":
            ready = ("l1", it) + make_l1_dense(it)
        else:
            ready = ("l2", it) + make_l2_dense(it, fuse_pool=(it == items - 1))

    # tail: the last item's dense phase has no agg left to hide in
    kind, itl, unit, fin = ready
    for nb in range(NB):
        unit(nb)
    for p in pool_q:
        pool_block(p)       # fills the finish_stats latency with PE work
    fin()
    pool_block(itl)

    es.close()


_CACHE = {}


def _get_nc(items, general):
    key = (items, general)
    if key not in _CACHE:
        nc = bacc.Bacc("TRN2", target_bir_lowering=False, debug=False,
                       num_devices=N_CORES)
        with tile.TileContext(nc) as tc:
            io = _declare_io(nc, items, general)
            _build_core(nc, tc, io, items, general)
        nc.compile()
        _CACHE[key] = nc
    return _CACHE[key]


def make_in_maps(A_hat, X, W1, b1, g1, beta1, W2, b2, g2, beta2,
                 Wa, ba, Wl, bl):
    """Host-side prep: shard over batch, scale+fp8+transpose A, fold gammas."""
    B = A_hat.shape[0]
    items = B // N_CORES
    general = bool(np.any(beta1 != 0) or np.any(beta2 != 0)
                   or np.any(g1 <= 0) or np.any(g2 <= 0))
    if general:
        w2f = np.asarray(W2, np.float32)
        waf = np.asarray(Wa, np.float32)
        wlf = np.asarray(Wl, np.float32)
    else:
        w2f = np.asarray(g1, np.float32)[:, None] * W2
        waf = (np.asarray(g2, np.float32)[:, None] * Wa).astype(np.float32)
        wlf = (np.asarray(g2, np.float32)[:, None] * Wl).astype(np.float32)
    w1f = np.asarray(W1, np.float32)
    shared = {
        "w1": (w1f / ASCALE).astype(bf16),
        "w2": (w2f / ASCALE).astype(bf16),
        "b1bc": np.ascontiguousarray(
            np.broadcast_to(np.asarray(b1, np.float32), (P, H))),
        "b2bc": np.ascontiguousarray(
            np.broadcast_to(np.asarray(b2, np.float32), (P, H))),
        "wa": waf, "wl": wlf,
        "ba": np.asarray(ba, np.float32).reshape(K, 1).copy(),
        "bl": np.asarray(bl, np.float32).reshape(K, 1).copy(),
        "ones": np.ones((P, 1), bf16),
    }
    if general:
        for nm, v in (("g1bc", g1), ("be1bc", beta1),
                      ("g2bc", g2), ("be2bc", beta2)):
            shared[nm] = np.ascontiguousarray(
                np.broadcast_to(np.asarray(v, np.float32), (P, H)))
    in_maps = []
    for c in range(N_CORES):
        m = dict(shared)
        Ab = np.asarray(A_hat[c * items:(c + 1) * items], np.float32)
        Af = (Ab * np.float32(ASCALE)).astype(f8e4)
        # at4[it, c2, p, k, n] = A^T[(2*c2+k)*128+p, n] * ASCALE
        at = Af.transpose(0, 2, 1).reshape(items, NB2, 2, P, N)
        m["at4"] = np.ascontiguousarray(at.transpose(0, 1, 3, 2, 4))
        Xb = np.asarray(X[c * items:(c + 1) * items], np.float32).astype(f8e4)
        # x4[it, p, cb, f] = X[cb*128+p, f]
        m["x4"] = np.ascontiguousarray(
            Xb.reshape(items, NB, P, F).transpose(0, 2, 1, 3))
        in_maps.append(m)
    return in_maps, items, general


def kernel(**inputs):
    in_maps, items, general = make_in_maps(**inputs)
    nc = _get_nc(items, general)
    res = run_bass_kernel_spmd(nc, in_maps, core_ids=list(range(N_CORES)))
    pred = np.concatenate([res.results[c]["op"] for c in range(N_CORES)], 0)
    logits = np.concatenate([res.results[c]["ol"] for c in range(N_CORES)], 0)
    return (np.asarray(pred, np.float32), np.asarray(logits, np.float32))


# revision 40
# speedup vs baseline: 1.0251x; 1.0251x over previous
"""TRN2 Bass kernel for nn_GCNBasic (2-layer GCN, B=32, N=2048, F=128, H=256).

Sharding: data-parallel over batch B across 8 NeuronCores (4 items/core);
small weights replicated.  A_hat is scaled by 2^17, cast to fp8-e4m3 and
transposed on the HOST (quarters HBM traffic vs f32); X and H1 are also
fp8, so BOTH aggregation matmuls (88% of the FLOPs) run in DoubleRow fp8
perf mode: each PE instruction contracts TWO 128-deep k-planes
(stationary [128,2,128] fp8, moving [128,2,512] fp8) at 2x bf16 MACs
(measured 212 ns per [128,512]-out instruction, LDWEIGHTS fully
overlapped with the previous matmul's stream).  The 2^-17 unscale is
folded into W1/W2 (bf16 holds tiny values exactly), so all LayerNorm
math is numerically identical to a bf16 kernel.  W2 stays bf16: an fp8
W2's quantization error is constant across nodes and survives the
mean-pool readout (measured 2.6e-2 final error vs 4.0e-3 with W2 bf16).

  (AX)^T[f,n]  = sum_c2 X[pair]-stationary  @ A^T[pair]   (fp8 DoubleRow,
                                              4 psum 512-chunks live)
  H1pre[n,h]   = (AX)^T[:,nb]-stat. @ W1/2^17 (bf16)
  H1           = relu(LN(H1pre + b1)) -> fp8  (bias-add+rowsum in one
                 DVE scalar_tensor_tensor with accum_out — this is also
                 the PSUM drain, so PSUM recycles after ~420ns; sumsq in
                 a second DVE stt; relu apply on ACT from SBUF staging)
  (AH)^T[hh,n] = sum_c2 H1[pair]-stationary  @ A^T[pair]   (fp8 DoubleRow)
  H2pre[n,k]   = sum_hh (AH)^T[hh,nb]-stat.  @ diag(g1)W2/2^17
  H2           = relu(LN(H2pre + b2)) -> bf16
  g            = 1^T H2 / N  (16 ones-STATIONARY matmuls into a [1,256]
                 PSUM row — weight loads are trivial — then 2 tiny PE
                 transposes give g^T halves for the f32 heads)

Items are software-pipelined at dense-matmul granularity: each dense
phase's 16 block-matmuls (+ their LN stats ops) drip into the NEXT
aggregation phase's c2 loop; finish_stats + applies run in quarters of
4 blocks so early applies overlap later stats; pool/head blocks trail
their LN2 applies by one phase (pool PSUM borrows an idle ps_big bank);
the last item fuses its mean-pool matmuls into the apply quarters.
A^T for item it+2 is prefetched a full phase ahead (pool_at holds 3
items, 12 MiB) so its ~11us HBM transfer never exposes.

gamma folds (diag(g1)@W2, diag(g2)@Wa/Wl) are exact because relu(g*z)=
g*relu(z) for g>0; beta==0 fast path (the problem's setup_inputs always
produces gamma=1, beta=0); a general gamma/beta path exists as a fallback.

Known TRN2 pitfalls worked around here: tensor_tensor_reduce crashes the
device; ACT/DVE writes into PSUM are unstable (reads are fine); Pool
(gpsimd) has no PSUM access and no TensorScalarPtr opcode; PSUM-resident
pre-activations gate the in-order PE queue on the slow apply chain, so
pre-activations are drained to SBUF by the bias-add stt instead.
"""

from contextlib import ExitStack

import numpy as np
import ml_dtypes

import concourse.bacc as bacc
import concourse.mybir as mybir
import concourse.tile as tile
from concourse.bass_utils import run_bass_kernel_spmd

F32 = mybir.dt.float32
BF16 = mybir.dt.bfloat16
FP16 = mybir.dt.float16
F8 = mybir.dt.float8e4
bf16 = ml_dtypes.bfloat16
f8e4 = ml_dtypes.float8_e4m3

N = 2048
F = 128
H = 256
K = 64
P = 128
NB = N // P
NB2 = NB // 2
NCH = N // 512
EPS = 1e-5
N_CORES = 8
ASCALE = 2.0 ** 17  # A_hat -> e4m3 range; 1/ASCALE folded into W1/W2
DR = mybir.MatmulPerfMode.DoubleRow


def _declare_io(nc, items, general):
    io = {}
    io["at4"] = nc.dram_tensor("at4", [items, NB2, P, 2, N], F8,
                               kind="ExternalInput")
    io["x4"] = nc.dram_tensor("x4", [items, P, NB, F], F8,
                              kind="ExternalInput")
    io["w1"] = nc.dram_tensor("w1", [F, H], BF16, kind="ExternalInput")
    io["w2"] = nc.dram_tensor("w2", [H, H], BF16, kind="ExternalInput")
    io["b1bc"] = nc.dram_tensor("b1bc", [P, H], F32, kind="ExternalInput")
    io["b2bc"] = nc.dram_tensor("b2bc", [P, H], F32, kind="ExternalInput")
    io["wa"] = nc.dram_tensor("wa", [H, K], F32, kind="ExternalInput")
    io["wl"] = nc.dram_tensor("wl", [H, K], F32, kind="ExternalInput")
    io["ba"] = nc.dram_tensor("ba", [K, 1], F32, kind="ExternalInput")
    io["bl"] = nc.dram_tensor("bl", [K, 1], F32, kind="ExternalInput")
    io["ones"] = nc.dram_tensor("ones", [P, 1], BF16, kind="ExternalInput")
    if general:
        io["g1bc"] = nc.dram_tensor("g1bc", [P, H], F32, kind="ExternalInput")
        io["be1bc"] = nc.dram_tensor("be1bc", [P, H], F32,
                                     kind="ExternalInput")
        io["g2bc"] = nc.dram_tensor("g2bc", [P, H], F32, kind="ExternalInput")
        io["be2bc"] = nc.dram_tensor("be2bc", [P, H], F32,
                                     kind="ExternalInput")
    io["op"] = nc.dram_tensor("op", [items, K], F32, kind="ExternalOutput")
    io["ol"] = nc.dram_tensor("ol", [items, K], F32, kind="ExternalOutput")
    return io


def _build_core(nc, tc, io, items, general):
    at4, x4 = io["at4"], io["x4"]
    es = ExitStack()

    consts = es.enter_context(tc.tile_pool(name="consts", bufs=1))
    wts = es.enter_context(tc.tile_pool(name="wts", bufs=1))
    pool_at = es.enter_context(tc.tile_pool(name="at", bufs=2 * NB2))
    pool_xb = es.enter_context(tc.tile_pool(name="xb", bufs=2))
    pool_axT = es.enter_context(tc.tile_pool(name="axT", bufs=2))
    pool_h1 = es.enter_context(tc.tile_pool(name="h1", bufs=2))
    pool_ahT = es.enter_context(tc.tile_pool(name="ahT", bufs=2))
    pool_h2 = es.enter_context(tc.tile_pool(name="h2", bufs=1))
    pool_hc = es.enter_context(tc.tile_pool(name="hc", bufs=NB))
    pool_sq = es.enter_context(tc.tile_pool(name="sq", bufs=2))
    pool_st = es.enter_context(tc.tile_pool(name="st", bufs=4))
    pool_gsb = es.enter_context(tc.tile_pool(name="gsb", bufs=4))
    pool_osb = es.enter_context(tc.tile_pool(name="osb", bufs=4))

    ps_big = es.enter_context(tc.tile_pool(name="ps_big", bufs=6, space="PSUM"))
    ps_h = es.enter_context(tc.tile_pool(name="ps_h", bufs=2, space="PSUM"))
    ps_sm = ps_h  # pool/head tiles share the dense-phase banks

    eps_t = consts.tile([P, 1], F32)
    nc.vector.memset(eps_t[:], EPS)
    id1 = consts.tile([1, 1], FP16)
    nc.vector.memset(id1[:], 1.0)
    ones_b = consts.tile([P, 1], BF16)
    w1_t = wts.tile([P, H], BF16)
    w2_t = [wts.tile([P, H], BF16, tag=f"w2_{hh}", name=f"w2_{hh}")
            for hh in range(2)]
    b1_t = wts.tile([P, H], F32)
    b2_t = wts.tile([P, H], F32)
    wa_t = [wts.tile([P, K], F32, tag=f"wa_{hh}", name=f"wa_{hh}")
            for hh in range(2)]
    wl_t = [wts.tile([P, K], F32, tag=f"wl_{hh}", name=f"wl_{hh}")
            for hh in range(2)]
    ba_t = wts.tile([K, 1], F32)
    bl_t = wts.tile([K, 1], F32)
    gb_t = {}
    if general:
        for nm in ("g1bc", "be1bc", "g2bc", "be2bc"):
            gb_t[nm] = wts.tile([P, H], F32, tag=nm, name=nm)

    def emit_weight_dmas():
        nc.sync.dma_start(ones_b[:], io["ones"][:])
        nc.sync.dma_start(w1_t[:], io["w1"][:])
        for hh in range(2):
            nc.sync.dma_start(w2_t[hh][:], io["w2"][hh * P:(hh + 1) * P, :])
        nc.sync.dma_start(b1_t[:], io["b1bc"][:])
        nc.sync.dma_start(b2_t[:], io["b2bc"][:])
        for hh in range(2):
            nc.sync.dma_start(wa_t[hh][:], io["wa"][hh * P:(hh + 1) * P, :])
            nc.sync.dma_start(wl_t[hh][:], io["wl"][hh * P:(hh + 1) * P, :])
        nc.sync.dma_start(ba_t[:], io["ba"][:])
        nc.sync.dma_start(bl_t[:], io["bl"][:])
        for nm, t in gb_t.items():
            nc.sync.dma_start(t[:], io[nm][:])

    inv_h = 1.0 / H

    # per-item live tiles (indexed by item)
    at_t = [None] * items
    xb_t = [None] * items
    axT_t = [None] * items
    h1_t = [None] * items
    ahT_t = [None] * items
    h2_t = [None] * items
    st1_t = [None] * items
    st2_t = [None] * items

    def load(it, chunks=1):
        xb = pool_xb.tile([P, NB, F], F8, tag="xb", name=f"xb_{it}")
        nc.sync.dma_start(xb[:], x4[it])
        xb_t[it] = xb
        ats = [pool_at.tile([P, 2, N], F8, tag="at", name=f"at_{it}_{c2}")
               for c2 in range(NB2)]
        cw = N // chunks
        for c2 in range(NB2):
            for k in range(chunks):
                nc.sync.dma_start(
                    ats[c2][:, :, k * cw:(k + 1) * cw],
                    at4[it, c2, :, :, k * cw:(k + 1) * cw])
        at_t[it] = ats

    def copy_out(dst, src, who):
        # PSUM->SBUF copy-outs alternate ACT/DVE (Pool cannot read PSUM)
        if who % 2 == 0:
            nc.scalar.copy(dst, src)
        else:
            nc.vector.tensor_copy(dst, src)

    def l1_agg(it, unit=None):
        # two pending dense units (PE matmul + LN stats) dripped per c2
        # iteration so the LN chain paces alongside pure agg matmuls
        at, xb = at_t[it], xb_t[it]
        pb = [ps_big.tile([P, 512], F32, tag="big", name=f"ax_{it}_{j}")
              for j in range(NCH)]
        for c2 in range(NB2):
            for j in range(NCH):
                nc.tensor.matmul(pb[j][:], xb[:, 2 * c2:2 * c2 + 2, :],
                                 at[c2][:, :, j * 512:(j + 1) * 512],
                                 start=(c2 == 0), stop=(c2 == NB2 - 1),
                                 perf_mode=DR)
            if unit is not None:
                unit(2 * c2)
                unit(2 * c2 + 1)
        axT = pool_axT.tile([P, N], BF16, tag="axT", name=f"axT_{it}")
        for j in range(NCH):
            copy_out(axT[:, j * 512:(j + 1) * 512], pb[j][:], j)
        axT_t[it] = axT

    def ln_stats(nb, ph, b_t, st, hc, sfx):
        # bias add + LN row-sum in one DVE op (accum_out; this is also
        # the PSUM drain so the in-order PE queue is only gated ~420ns);
        # sum column H comes from the matmul's extra W column; sumsq in
        # a second DVE stt; ACT stays free for the applies
        nc.vector.scalar_tensor_tensor(
            out=hc[:], in0=ph[:, 0:H], scalar=1.0, in1=b_t[:],
            op0=mybir.AluOpType.mult, op1=mybir.AluOpType.add,
            accum_out=st[:, 0, nb:nb + 1])
        sq = pool_sq.tile([P, H], F32, tag="sq", name=f"sq_{sfx}")
        nc.vector.scalar_tensor_tensor(
            out=sq[:], in0=hc[:], scalar=1.0, in1=hc[:],
            op0=mybir.AluOpType.mult, op1=mybir.AluOpType.mult,
            accum_out=st[:, 1, nb:nb + 1])

    def finish_stats(st, lo=0, hi=NB):
        # runs on a [lo:hi) slice of the nb columns so early blocks'
        # applies overlap later blocks' stats (subtile deps)
        s = st[:, :, lo:hi]
        nc.vector.tensor_scalar(out=s[:, 2, :], in0=s[:, 0, :],
                                scalar1=-inv_h, scalar2=None,
                                op0=mybir.AluOpType.mult)          # -mu
        nc.vector.tensor_tensor(out=s[:, 3, :], in0=s[:, 2, :], in1=s[:, 2, :],
                                op=mybir.AluOpType.mult)           # mu^2
        nc.vector.tensor_scalar(out=s[:, 4, :], in0=s[:, 1, :],
                                scalar1=inv_h, scalar2=None,
                                op0=mybir.AluOpType.mult)          # E[x^2]
        nc.vector.tensor_tensor(out=s[:, 4, :], in0=s[:, 4, :], in1=s[:, 3, :],
                                op=mybir.AluOpType.subtract)       # var
        nc.scalar.activation(out=s[:, 5, :], in_=s[:, 4, :],
                             func=mybir.ActivationFunctionType.Sqrt,
                             bias=eps_t[:], scale=1.0)             # sd
        nc.vector.reciprocal(out=s[:, 6, :], in_=s[:, 5, :])       # 1/sd
        nc.vector.tensor_tensor(out=s[:, 7, :], in0=s[:, 2, :], in1=s[:, 6, :],
                                op=mybir.AluOpType.mult)           # -mu/sd

    def apply_ln(nb, hc, st, h_out, g_bc, be_bc):
        if not general:
            nc.scalar.activation(out=h_out, in_=hc[:],
                                 func=mybir.ActivationFunctionType.Relu,
                                 bias=st[:, 7, nb:nb + 1],
                                 scale=st[:, 6, nb:nb + 1])
        else:
            nc.scalar.activation(out=hc[:], in_=hc[:],
                                 func=mybir.ActivationFunctionType.Identity,
                                 bias=st[:, 7, nb:nb + 1],
                                 scale=st[:, 6, nb:nb + 1])
            nc.gpsimd.tensor_tensor(out=hc[:], in0=hc[:], in1=g_bc[:],
                                    op=mybir.AluOpType.mult)
            nc.vector.tensor_tensor(out=hc[:], in0=hc[:], in1=be_bc[:],
                                    op=mybir.AluOpType.add)
            nc.scalar.activation(out=h_out, in_=hc[:],
                                 func=mybir.ActivationFunctionType.Relu)

    def l2_agg(it, unit=None):
        at, h1 = at_t[it], h1_t[it]
        ahT = [pool_ahT.tile([P, N], BF16, tag="ahT", name=f"ahT_{it}_{hh}")
               for hh in range(2)]
        ucnt = 0
        for hh in range(2):
            pb = [ps_big.tile([P, 512], F32, tag="big",
                              name=f"ah_{it}_{hh}_{j}") for j in range(NCH)]
            for c2 in range(NB2):
                for j in range(NCH):
                    nc.tensor.matmul(
                        pb[j][:], h1[:, 2 * c2:2 * c2 + 2, hh * P:(hh + 1) * P],
                        at[c2][:, :, j * 512:(j + 1) * 512],
                        start=(c2 == 0), stop=(c2 == NB2 - 1), perf_mode=DR)
                if unit is not None:
                    unit(ucnt)
                    ucnt += 1
            for j in range(NCH):
                copy_out(ahT[hh][:, j * 512:(j + 1) * 512], pb[j][:],
                         j + hh)
        ahT_t[it] = ahT

    pg_t = [None] * items

    def make_l1_dense(it):
        axT = axT_t[it]
        st1 = pool_st.tile([P, 8, NB], F32, tag="st", name=f"st1_{it}")
        h1 = pool_h1.tile([P, NB, H], F8, tag="h1", name=f"h1_{it}")
        hc1 = []
        st1_t[it], h1_t[it] = st1, h1

        def unit(nb):
            ph = ps_h.tile([P, H], F32, tag="h", name=f"p1_{it}_{nb}")
            nc.tensor.matmul(ph[:], axT[:, nb * P:(nb + 1) * P], w1_t[:],
                             start=True, stop=True)
            hc = pool_hc.tile([P, H], F32, tag="hc", name=f"hc1_{it}_{nb}")
            ln_stats(nb, ph, b1_t, st1, hc, f"1_{it}_{nb}")
            hc1.append(hc)

        def fin():
            for q in range(0, NB, 4):
                finish_stats(st1, q, q + 4)
                for nb in range(q, q + 4):
                    apply_ln(nb, hc1[nb], st1, h1[:, nb, :],
                             gb_t.get("g1bc"), gb_t.get("be1bc"))

        return unit, fin

    def make_l2_dense(it, fuse_pool=False):
        ahT = ahT_t[it]
        st2 = pool_st.tile([P, 8, NB], F32, tag="st", name=f"st2_{it}")
        h2 = pool_h2.tile([P, NB, H], BF16, tag="h2", name=f"h2_{it}")
        hc2 = []
        st2_t[it], h2_t[it] = st2, h2

        def unit(nb):
            ph = ps_h.tile([P, H], F32, tag="h", name=f"p2_{it}_{nb}")
            for hh in range(2):
                nc.tensor.matmul(ph[:], ahT[hh][:, nb * P:(nb + 1) * P],
                                 w2_t[hh][:], start=(hh == 0), stop=(hh == 1))
            hc = pool_hc.tile([P, H], F32, tag="hc", name=f"hc2_{it}_{nb}")
            ln_stats(nb, ph, b2_t, st2, hc, f"2_{it}_{nb}")
            hc2.append(hc)

        def fin():
            if fuse_pool:
                # last item: mean-pool accumulation rides the apply
                # quarters so the tail chain stays short
                pg = [ps_sm.tile([P, 1], F32, tag="h", name=f"pg_{it}_{kh}")
                      for kh in range(2)]
                pg_t[it] = pg
            for q in range(0, NB, 4):
                finish_stats(st2, q, q + 4)
                for nb in range(q, q + 4):
                    apply_ln(nb, hc2[nb], st2, h2[:, nb, :],
                             gb_t.get("g2bc"), gb_t.get("be2bc"))
                    if fuse_pool:
                        for kh in range(2):
                            nc.tensor.matmul(pg[kh][:],
                                             h2[:, nb, kh * P:(kh + 1) * P],
                                             ones_b[:], start=(nb == 0),
                                             stop=(nb == NB - 1))

        return unit, fin

    def pool_block(it):
        h2 = h2_t[it]
        gsb = pool_gsb.tile([P, 2], F32, tag="g", name=f"g_{it}")
        if pg_t[it] is None:
            pg = [ps_sm.tile([P, 1], F32, tag="h", name=f"pg_{it}_{kh}")
                  for kh in range(2)]
            for nb in range(NB):
                for kh in range(2):
                    nc.tensor.matmul(pg[kh][:],
                                     h2[:, nb, kh * P:(kh + 1) * P],
                                     ones_b[:], start=(nb == 0),
                                     stop=(nb == NB - 1))
        else:
            pg = pg_t[it]
        for kh in range(2):
            nc.scalar.mul(gsb[:, kh:kh + 1], pg[kh][:], 1.0 / N)

        for hd, (w_t, b_t, out_d) in enumerate(
                ((wa_t, ba_t, io["op"]), (wl_t, bl_t, io["ol"]))):
            po = ps_sm.tile([K, 1], F32, tag="h", name=f"po_{it}_{hd}")
            for kh in range(2):
                nc.tensor.matmul(po[:], w_t[kh][:], gsb[:, kh:kh + 1],
                                 start=(kh == 0), stop=(kh == 1))
            osb = pool_osb.tile([K, 1], F32, tag="o", name=f"o_{it}_{hd}")
            nc.scalar.activation(out=osb[:], in_=po[:],
                                 func=mybir.ActivationFunctionType.Identity,
                                 bias=b_t[:], scale=1.0)
            nc.sync.dma_start(out_d[it:it + 1, :], osb[:])

    # ---- software pipeline: dense phases interleave into the next agg
    # phase's c2 loop; pool/head blocks trail their applies by one phase ----
    load(0, chunks=4)       # chunked so the first tiles land early
    emit_weight_dmas()
    if items == 1:
        l1_agg(0)
        unit, fin = make_l1_dense(0)
        for nb in range(NB):
            unit(nb)
        fin()
        l2_agg(0)
        unit, fin = make_l2_dense(0, fuse_pool=True)
        for nb in range(NB):
            unit(nb)
        fin()
        pool_block(0)
        es.close()
        return

    load(1, chunks=2)
    phases = [("l1", 0), ("l1", 1)]
    for it in range(items):
        phases.append(("l2", it))
        if it + 2 < items:
            phases.append(("l1", it + 2))

    ready = None            # (kind, it, unit, fin) pending dense phase
    pool_q = []             # items whose pool block is due next phase
    for kind, it in phases:
        pool_now, pool_q = pool_q, []
        cur, ready = ready, None
        unit = cur[2] if cur else None
        if kind == "l1":
            if it >= 2:
                load(it, chunks=2)
            l1_agg(it, unit)
        else:
            l2_agg(it, unit)
        if cur is not None:
            cur[3]()
            if cur[0] == "l2":
                pool_q.append(cur[1])
        for p in pool_now:
            pool_block(p)
        if kind == "l1":
            ready = ("l1", it) + make_l1_dense(it)
        else:
            ready = ("l2", it) + make_l2_dense(it, fuse_pool=(it == items - 1))

    # tail: the last item's dense phase has no agg left to hide in
    kind, itl, unit, fin = ready
    for nb in range(NB):
        unit(nb)
    for p in pool_q:
        pool_block(p)       # fills the finish_stats latency with PE work
    fin()
    pool_block(itl)

    es.close()


_CACHE = {}


def _get_nc(items, general):
    key = (items, general)
    if key not in _CACHE:
        nc = bacc.Bacc("TRN2", target_bir_lowering=False, debug=False,
                       num_devices=N_CORES)
        with tile.TileContext(nc) as tc:
            io = _declare_io(nc, items, general)
            _build_core(nc, tc, io, items, general)
        nc.compile()
        _CACHE[key] = nc
    return _CACHE[key]


def make_in_maps(A_hat, X, W1, b1, g1, beta1, W2, b2, g2, beta2,
                 Wa, ba, Wl, bl):
    """Host-side prep: shard over batch, scale+fp8+transpose A, fold gammas."""
    B = A_hat.shape[0]
    items = B // N_CORES
    general = bool(np.any(beta1 != 0) or np.any(beta2 != 0)
                   or np.any(g1 <= 0) or np.any(g2 <= 0))
    if general:
        w2f = np.asarray(W2, np.float32)
        waf = np.asarray(Wa, np.float32)
        wlf = np.asarray(Wl, np.float32)
    else:
        w2f = np.asarray(g1, np.float32)[:, None] * W2
        waf = (np.asarray(g2, np.float32)[:, None] * Wa).astype(np.float32)
        wlf = (np.asarray(g2, np.float32)[:, None] * Wl).astype(np.float32)
    w1f = np.asarray(W1, np.float32)
    shared = {
        "w1": (w1f / ASCALE).astype(bf16),
        "w2": (w2f / ASCALE).astype(bf16),
        "b1bc": np.ascontiguousarray(
            np.broadcast_to(np.asarray(b1, np.float32), (P, H))),
        "b2bc": np.ascontiguousarray(
            np.broadcast_to(np.asarray(b2, np.float32), (P, H))),
        "wa": waf, "wl": wlf,
        "ba": np.asarray(ba, np.float32).reshape(K, 1).copy(),
        "bl": np.asarray(bl, np.float32).reshape(K, 1).copy(),
        "ones": np.ones((P, 1), bf16),
    }
    if general:
        for nm, v in (("g1bc", g1), ("be1bc", beta1),
                      ("g2bc", g2), ("be2bc", beta2)):
            shared[nm] = np.ascontiguousarray(
                np.broadcast_to(np.asarray(v, np.float32), (P, H)))
    in_maps = []
    for c in range(N_CORES):
        m = dict(shared)
        Ab = np.asarray(A_hat[c * items:(c + 1) * items], np.float32)
        Af = (Ab * np.float32(ASCALE)).astype(f8e4)
        # at4[it, c2, p, k, n] = A^T[(2*c2+k)*128+p, n] * ASCALE
        at = Af.transpose(0, 2, 1).reshape(items, NB2, 2, P, N)
        m["at4"] = np.ascontiguousarray(at.transpose(0, 1, 3, 2, 4))
        Xb = np.asarray(X[c * items:(c + 1) * items], np.float32).astype(f8e4)
        # x4[it, p, cb, f] = X[cb*128+p, f]
        m["x4"] = np.ascontiguousarray(
            Xb.reshape(items, NB, P, F).transpose(0, 2, 1, 3))
        in_maps.append(m)
    return in_maps, items, general


def kernel(**inputs):
    in_maps, items, general = make_in_maps(**inputs)
    nc = _get_nc(items, general)
    res = run_bass_kernel_spmd(nc, in_maps, core_ids=list(range(N_CORES)))
    pred = np.concatenate([res.results[c]["op"] for c in range(N_CORES)], 0)
    logits = np.concatenate([res.results[c]["ol"] for c in range(N_CORES)], 0)
    return (np.asarray(pred, np.float32), np.asarray(logits, np.float32))


# revision 41
# speedup vs baseline: 1.0334x; 1.0081x over previous
"""TRN2 Bass kernel for nn_GCNBasic (2-layer GCN, B=32, N=2048, F=128, H=256).

Sharding: data-parallel over batch B across 8 NeuronCores (4 items/core);
small weights replicated.  A_hat is scaled by 2^17, cast to fp8-e4m3 and
transposed on the HOST (quarters HBM traffic vs f32); X and H1 are also
fp8, so BOTH aggregation matmuls (88% of the FLOPs) run in DoubleRow fp8
perf mode: each PE instruction contracts TWO 128-deep k-planes
(stationary [128,2,128] fp8, moving [128,2,512] fp8) at 2x bf16 MACs
(measured 212 ns per [128,512]-out instruction, LDWEIGHTS fully
overlapped with the previous matmul's stream).  The 2^-17 unscale is
folded into W1/W2 (bf16 holds tiny values exactly), so all LayerNorm
math is numerically identical to a bf16 kernel.  W2 stays bf16: an fp8
W2's quantization error is constant across nodes and survives the
mean-pool readout (measured 2.6e-2 final error vs 4.0e-3 with W2 bf16).

  (AX)^T[f,n]  = sum_c2 X[pair]-stationary  @ A^T[pair]   (fp8 DoubleRow,
                                              4 psum 512-chunks live)
  H1pre[n,h]   = (AX)^T[:,nb]-stat. @ W1/2^17 (bf16)
  H1           = relu(LN(H1pre + b1)) -> fp8  (bias-add+rowsum in one
                 DVE scalar_tensor_tensor with accum_out — this is also
                 the PSUM drain, so PSUM recycles after ~420ns; sumsq in
                 a second DVE stt; relu apply on ACT from SBUF staging)
  (AH)^T[hh,n] = sum_c2 H1[pair]-stationary  @ A^T[pair]   (fp8 DoubleRow)
  H2pre[n,k]   = sum_hh (AH)^T[hh,nb]-stat.  @ diag(g1)W2/2^17
  H2           = relu(LN(H2pre + b2)) -> bf16
  g^T          = sum_nb H2[nb,kh]-stationary @ ones  (mean pool via PE)
  outputs      = diag(g2)Wa/Wl heads in fp32, biases added on ACT.

Items are software-pipelined at dense-matmul granularity: each dense
phase's 16 block-matmuls (+ their LN stats ops) drip two-per-c2 into
the NEXT aggregation phase's c2 loop; finish_stats + applies run in
quarters of 4 blocks so early applies overlap later stats; pool/head
blocks trail their LN2 applies by one phase; the last item fuses its
mean-pool accumulation into the apply quarters.  Scheduling findings
(measured): burst-dripping 4 units stalls the 2-deep ph rotation;
chunks>4 per A-tile DMA floods the SP sequencer (~565ns per issue);
3-item A prefetch and PSUM 5/3 rebalance were neutral-to-negative.

gamma folds (diag(g1)@W2, diag(g2)@Wa/Wl) are exact because relu(g*z)=
g*relu(z) for g>0; beta==0 fast path (the problem's setup_inputs always
produces gamma=1, beta=0); a general gamma/beta path exists as a fallback.

Known TRN2 pitfalls worked around here: tensor_tensor_reduce crashes the
device; ACT/DVE writes into PSUM are unstable (reads are fine); Pool
(gpsimd) has no PSUM access and no TensorScalarPtr opcode; PSUM-resident
pre-activations gate the in-order PE queue on the slow apply chain, so
pre-activations are drained to SBUF by the bias-add stt instead.
"""

from contextlib import ExitStack

import numpy as np
import ml_dtypes

import concourse.bacc as bacc
import concourse.mybir as mybir
import concourse.tile as tile
from concourse.bass_utils import run_bass_kernel_spmd

F32 = mybir.dt.float32
BF16 = mybir.dt.bfloat16
FP16 = mybir.dt.float16
F8 = mybir.dt.float8e4
bf16 = ml_dtypes.bfloat16
f8e4 = ml_dtypes.float8_e4m3

N = 2048
F = 128
H = 256
K = 64
P = 128
NB = N // P
NB2 = NB // 2
NCH = N // 512
EPS = 1e-5
N_CORES = 8
ASCALE = 2.0 ** 17  # A_hat -> e4m3 range; 1/ASCALE folded into W1/W2
DR = mybir.MatmulPerfMode.DoubleRow


def _declare_io(nc, items, general):
    io = {}
    io["at4"] = nc.dram_tensor("at4", [items, NB2, P, 2, N], F8,
                               kind="ExternalInput")
    io["x4"] = nc.dram_tensor("x4", [items, P, NB, F], F8,
                              kind="ExternalInput")
    io["w1"] = nc.dram_tensor("w1", [F, H], BF16, kind="ExternalInput")
    io["w2"] = nc.dram_tensor("w2", [H, H], BF16, kind="ExternalInput")
    io["b1bc"] = nc.dram_tensor("b1bc", [P, H], F32, kind="ExternalInput")
    io["b2bc"] = nc.dram_tensor("b2bc", [P, H], F32, kind="ExternalInput")
    io["wa"] = nc.dram_tensor("wa", [H, K], F32, kind="ExternalInput")
    io["wl"] = nc.dram_tensor("wl", [H, K], F32, kind="ExternalInput")
    io["ba"] = nc.dram_tensor("ba", [K, 1], F32, kind="ExternalInput")
    io["bl"] = nc.dram_tensor("bl", [K, 1], F32, kind="ExternalInput")
    io["ones"] = nc.dram_tensor("ones", [P, 1], BF16, kind="ExternalInput")
    if general:
        io["g1bc"] = nc.dram_tensor("g1bc", [P, H], F32, kind="ExternalInput")
        io["be1bc"] = nc.dram_tensor("be1bc", [P, H], F32,
                                     kind="ExternalInput")
        io["g2bc"] = nc.dram_tensor("g2bc", [P, H], F32, kind="ExternalInput")
        io["be2bc"] = nc.dram_tensor("be2bc", [P, H], F32,
                                     kind="ExternalInput")
    io["op"] = nc.dram_tensor("op", [items, K], F32, kind="ExternalOutput")
    io["ol"] = nc.dram_tensor("ol", [items, K], F32, kind="ExternalOutput")
    return io


def _build_core(nc, tc, io, items, general):
    at4, x4 = io["at4"], io["x4"]
    es = ExitStack()

    consts = es.enter_context(tc.tile_pool(name="consts", bufs=1))
    wts = es.enter_context(tc.tile_pool(name="wts", bufs=1))
    pool_at = es.enter_context(tc.tile_pool(name="at", bufs=2 * NB2))
    pool_xb = es.enter_context(tc.tile_pool(name="xb", bufs=2))
    pool_axT = es.enter_context(tc.tile_pool(name="axT", bufs=2))
    pool_h1 = es.enter_context(tc.tile_pool(name="h1", bufs=2))
    pool_ahT = es.enter_context(tc.tile_pool(name="ahT", bufs=2))
    pool_h2 = es.enter_context(tc.tile_pool(name="h2", bufs=1))
    pool_hc = es.enter_context(tc.tile_pool(name="hc", bufs=NB))
    pool_sq = es.enter_context(tc.tile_pool(name="sq", bufs=2))
    pool_st = es.enter_context(tc.tile_pool(name="st", bufs=4))
    pool_gsb = es.enter_context(tc.tile_pool(name="gsb", bufs=4))
    pool_osb = es.enter_context(tc.tile_pool(name="osb", bufs=4))

    ps_big = es.enter_context(tc.tile_pool(name="ps_big", bufs=6, space="PSUM"))
    ps_h = es.enter_context(tc.tile_pool(name="ps_h", bufs=2, space="PSUM"))
    ps_sm = ps_h  # pool/head tiles share the dense-phase banks

    eps_t = consts.tile([P, 1], F32)
    nc.vector.memset(eps_t[:], EPS)
    id1 = consts.tile([1, 1], FP16)
    nc.vector.memset(id1[:], 1.0)
    ones_b = consts.tile([P, 1], BF16)
    w1_t = wts.tile([P, H], BF16)
    w2_t = [wts.tile([P, H], BF16, tag=f"w2_{hh}", name=f"w2_{hh}")
            for hh in range(2)]
    b1_t = wts.tile([P, H], F32)
    b2_t = wts.tile([P, H], F32)
    wa_t = [wts.tile([P, K], F32, tag=f"wa_{hh}", name=f"wa_{hh}")
            for hh in range(2)]
    wl_t = [wts.tile([P, K], F32, tag=f"wl_{hh}", name=f"wl_{hh}")
            for hh in range(2)]
    ba_t = wts.tile([K, 1], F32)
    bl_t = wts.tile([K, 1], F32)
    gb_t = {}
    if general:
        for nm in ("g1bc", "be1bc", "g2bc", "be2bc"):
            gb_t[nm] = wts.tile([P, H], F32, tag=nm, name=nm)

    def emit_weight_dmas():
        nc.sync.dma_start(ones_b[:], io["ones"][:])
        nc.sync.dma_start(w1_t[:], io["w1"][:])
        for hh in range(2):
            nc.sync.dma_start(w2_t[hh][:], io["w2"][hh * P:(hh + 1) * P, :])
        nc.sync.dma_start(b1_t[:], io["b1bc"][:])
        nc.sync.dma_start(b2_t[:], io["b2bc"][:])
        for hh in range(2):
            nc.sync.dma_start(wa_t[hh][:], io["wa"][hh * P:(hh + 1) * P, :])
            nc.sync.dma_start(wl_t[hh][:], io["wl"][hh * P:(hh + 1) * P, :])
        nc.sync.dma_start(ba_t[:], io["ba"][:])
        nc.sync.dma_start(bl_t[:], io["bl"][:])
        for nm, t in gb_t.items():
            nc.sync.dma_start(t[:], io[nm][:])

    inv_h = 1.0 / H

    # per-item live tiles (indexed by item)
    at_t = [None] * items
    xb_t = [None] * items
    axT_t = [None] * items
    h1_t = [None] * items
    ahT_t = [None] * items
    h2_t = [None] * items
    st1_t = [None] * items
    st2_t = [None] * items

    def load(it, chunks=1):
        xb = pool_xb.tile([P, NB, F], F8, tag="xb", name=f"xb_{it}")
        nc.sync.dma_start(xb[:], x4[it])
        xb_t[it] = xb
        ats = [pool_at.tile([P, 2, N], F8, tag="at", name=f"at_{it}_{c2}")
               for c2 in range(NB2)]
        cw = N // chunks
        for c2 in range(NB2):
            for k in range(chunks):
                nc.sync.dma_start(
                    ats[c2][:, :, k * cw:(k + 1) * cw],
                    at4[it, c2, :, :, k * cw:(k + 1) * cw])
        at_t[it] = ats

    def copy_out(dst, src, who):
        # PSUM->SBUF copy-outs alternate ACT/DVE (Pool cannot read PSUM)
        if who % 2 == 0:
            nc.scalar.copy(dst, src)
        else:
            nc.vector.tensor_copy(dst, src)

    def l1_agg(it, unit=None):
        # two pending dense units (PE matmul + LN stats) dripped per c2
        # iteration so the LN chain paces alongside pure agg matmuls
        at, xb = at_t[it], xb_t[it]
        pb = [ps_big.tile([P, 512], F32, tag="big", name=f"ax_{it}_{j}")
              for j in range(NCH)]
        for c2 in range(NB2):
            for j in range(NCH):
                nc.tensor.matmul(pb[j][:], xb[:, 2 * c2:2 * c2 + 2, :],
                                 at[c2][:, :, j * 512:(j + 1) * 512],
                                 start=(c2 == 0), stop=(c2 == NB2 - 1),
                                 perf_mode=DR)
            if unit is not None:
                unit(2 * c2)
                unit(2 * c2 + 1)
        axT = pool_axT.tile([P, N], BF16, tag="axT", name=f"axT_{it}")
        for j in range(NCH):
            copy_out(axT[:, j * 512:(j + 1) * 512], pb[j][:], j)
        axT_t[it] = axT

    def ln_stats(nb, ph, b_t, st, hc, sfx):
        # bias add + LN row-sum in one DVE op (accum_out; this is also
        # the PSUM drain so the in-order PE queue is only gated ~420ns);
        # sum column H comes from the matmul's extra W column; sumsq in
        # a second DVE stt; ACT stays free for the applies
        nc.vector.scalar_tensor_tensor(
            out=hc[:], in0=ph[:, 0:H], scalar=1.0, in1=b_t[:],
            op0=mybir.AluOpType.mult, op1=mybir.AluOpType.add,
            accum_out=st[:, 0, nb:nb + 1])
        sq = pool_sq.tile([P, H], F32, tag="sq", name=f"sq_{sfx}")
        nc.vector.scalar_tensor_tensor(
            out=sq[:], in0=hc[:], scalar=1.0, in1=hc[:],
            op0=mybir.AluOpType.mult, op1=mybir.AluOpType.mult,
            accum_out=st[:, 1, nb:nb + 1])

    def finish_stats(st, lo=0, hi=NB):
        # runs on a [lo:hi) slice of the nb columns so early blocks'
        # applies overlap later blocks' stats (subtile deps)
        s = st[:, :, lo:hi]
        nc.vector.tensor_scalar(out=s[:, 2, :], in0=s[:, 0, :],
                                scalar1=-inv_h, scalar2=None,
                                op0=mybir.AluOpType.mult)          # -mu
        nc.vector.tensor_tensor(out=s[:, 3, :], in0=s[:, 2, :], in1=s[:, 2, :],
                                op=mybir.AluOpType.mult)           # mu^2
        nc.vector.tensor_scalar(out=s[:, 4, :], in0=s[:, 1, :],
                                scalar1=inv_h, scalar2=None,
                                op0=mybir.AluOpType.mult)          # E[x^2]
        nc.vector.tensor_tensor(out=s[:, 4, :], in0=s[:, 4, :], in1=s[:, 3, :],
                                op=mybir.AluOpType.subtract)       # var
        nc.scalar.activation(out=s[:, 5, :], in_=s[:, 4, :],
                             func=mybir.ActivationFunctionType.Sqrt,
                             bias=eps_t[:], scale=1.0)             # sd
        nc.vector.reciprocal(out=s[:, 6, :], in_=s[:, 5, :])       # 1/sd
        nc.vector.tensor_tensor(out=s[:, 7, :], in0=s[:, 2, :], in1=s[:, 6, :],
                                op=mybir.AluOpType.mult)           # -mu/sd

    def apply_ln(nb, hc, st, h_out, g_bc, be_bc):
        if not general:
            nc.scalar.activation(out=h_out, in_=hc[:],
                                 func=mybir.ActivationFunctionType.Relu,
                                 bias=st[:, 7, nb:nb + 1],
                                 scale=st[:, 6, nb:nb + 1])
        else:
            nc.scalar.activation(out=hc[:], in_=hc[:],
                                 func=mybir.ActivationFunctionType.Identity,
                                 bias=st[:, 7, nb:nb + 1],
                                 scale=st[:, 6, nb:nb + 1])
            nc.gpsimd.tensor_tensor(out=hc[:], in0=hc[:], in1=g_bc[:],
                                    op=mybir.AluOpType.mult)
            nc.vector.tensor_tensor(out=hc[:], in0=hc[:], in1=be_bc[:],
                                    op=mybir.AluOpType.add)
            nc.scalar.activation(out=h_out, in_=hc[:],
                                 func=mybir.ActivationFunctionType.Relu)

    def l2_agg(it, unit=None):
        at, h1 = at_t[it], h1_t[it]
        ahT = [pool_ahT.tile([P, N], BF16, tag="ahT", name=f"ahT_{it}_{hh}")
               for hh in range(2)]
        ucnt = 0
        for hh in range(2):
            pb = [ps_big.tile([P, 512], F32, tag="big",
                              name=f"ah_{it}_{hh}_{j}") for j in range(NCH)]
            for c2 in range(NB2):
                for j in range(NCH):
                    nc.tensor.matmul(
                        pb[j][:], h1[:, 2 * c2:2 * c2 + 2, hh * P:(hh + 1) * P],
                        at[c2][:, :, j * 512:(j + 1) * 512],
                        start=(c2 == 0), stop=(c2 == NB2 - 1), perf_mode=DR)
                if unit is not None:
                    unit(ucnt)
                    ucnt += 1
            for j in range(NCH):
                copy_out(ahT[hh][:, j * 512:(j + 1) * 512], pb[j][:],
                         j + hh)
        ahT_t[it] = ahT

    pg_t = [None] * items

    def make_l1_dense(it):
        axT = axT_t[it]
        st1 = pool_st.tile([P, 8, NB], F32, tag="st", name=f"st1_{it}")
        h1 = pool_h1.tile([P, NB, H], F8, tag="h1", name=f"h1_{it}")
        hc1 = []
        st1_t[it], h1_t[it] = st1, h1

        def unit(nb):
            ph = ps_h.tile([P, H], F32, tag="h", name=f"p1_{it}_{nb}")
            nc.tensor.matmul(ph[:], axT[:, nb * P:(nb + 1) * P], w1_t[:],
                             start=True, stop=True)
            hc = pool_hc.tile([P, H], F32, tag="hc", name=f"hc1_{it}_{nb}")
            ln_stats(nb, ph, b1_t, st1, hc, f"1_{it}_{nb}")
            hc1.append(hc)

        def fin():
            for q in range(0, NB, 4):
                finish_stats(st1, q, q + 4)
                for nb in range(q, q + 4):
                    apply_ln(nb, hc1[nb], st1, h1[:, nb, :],
                             gb_t.get("g1bc"), gb_t.get("be1bc"))

        return unit, fin

    def make_l2_dense(it, fuse_pool=False):
        ahT = ahT_t[it]
        st2 = pool_st.tile([P, 8, NB], F32, tag="st", name=f"st2_{it}")
        h2 = pool_h2.tile([P, NB, H], BF16, tag="h2", name=f"h2_{it}")
        hc2 = []
        st2_t[it], h2_t[it] = st2, h2

        def unit(nb):
            ph = ps_h.tile([P, H], F32, tag="h", name=f"p2_{it}_{nb}")
            for hh in range(2):
                nc.tensor.matmul(ph[:], ahT[hh][:, nb * P:(nb + 1) * P],
                                 w2_t[hh][:], start=(hh == 0), stop=(hh == 1))
            hc = pool_hc.tile([P, H], F32, tag="hc", name=f"hc2_{it}_{nb}")
            ln_stats(nb, ph, b2_t, st2, hc, f"2_{it}_{nb}")
            hc2.append(hc)

        def fin():
            if fuse_pool:
                # last item: mean-pool accumulation rides the apply
                # quarters so the tail chain stays short
                pg = [ps_sm.tile([P, 1], F32, tag="h", name=f"pg_{it}_{kh}")
                      for kh in range(2)]
                pg_t[it] = pg
            for q in range(0, NB, 4):
                finish_stats(st2, q, q + 4)
                for nb in range(q, q + 4):
                    apply_ln(nb, hc2[nb], st2, h2[:, nb, :],
                             gb_t.get("g2bc"), gb_t.get("be2bc"))
                    if fuse_pool:
                        for kh in range(2):
                            nc.tensor.matmul(pg[kh][:],
                                             h2[:, nb, kh * P:(kh + 1) * P],
                                             ones_b[:], start=(nb == 0),
                                             stop=(nb == NB - 1))

        return unit, fin

    def pool_block(it):
        h2 = h2_t[it]
        gsb = pool_gsb.tile([P, 2], F32, tag="g", name=f"g_{it}")
        if pg_t[it] is None:
            pg = [ps_sm.tile([P, 1], F32, tag="h", name=f"pg_{it}_{kh}")
                  for kh in range(2)]
            for nb in range(NB):
                for kh in range(2):
                    nc.tensor.matmul(pg[kh][:],
                                     h2[:, nb, kh * P:(kh + 1) * P],
                                     ones_b[:], start=(nb == 0),
                                     stop=(nb == NB - 1))
        else:
            pg = pg_t[it]
        for kh in range(2):
            nc.scalar.mul(gsb[:, kh:kh + 1], pg[kh][:], 1.0 / N)

        for hd, (w_t, b_t, out_d) in enumerate(
                ((wa_t, ba_t, io["op"]), (wl_t, bl_t, io["ol"]))):
            po = ps_sm.tile([K, 1], F32, tag="h", name=f"po_{it}_{hd}")
            for kh in range(2):
                nc.tensor.matmul(po[:], w_t[kh][:], gsb[:, kh:kh + 1],
                                 start=(kh == 0), stop=(kh == 1))
            osb = pool_osb.tile([K, 1], F32, tag="o", name=f"o_{it}_{hd}")
            nc.scalar.activation(out=osb[:], in_=po[:],
                                 func=mybir.ActivationFunctionType.Identity,
                                 bias=b_t[:], scale=1.0)
            nc.sync.dma_start(out_d[it:it + 1, :], osb[:])

    # ---- software pipeline: dense phases interleave into the next agg
    # phase's c2 loop; pool/head blocks trail their applies by one phase ----
    load(0, chunks=4)       # chunked so the first tiles land early
    emit_weight_dmas()
    if items == 1:
        l1_agg(0)
        unit, fin = make_l1_dense(0)
        for nb in range(NB):
            unit(nb)
        fin()
        l2_agg(0)
        unit, fin = make_l2_dense(0, fuse_pool=True)
        for nb in range(NB):
            unit(nb)
        fin()
        pool_block(0)
        es.close()
        return

    load(1, chunks=2)
    phases = [("l1", 0), ("l1", 1)]
    for it in range(items):
        phases.append(("l2", it))
        if it + 2 < items:
            phases.append(("l1", it + 2))

    ready = None            # (kind, it, unit, fin) pending dense phase
    pool_q = []             # items whose pool block is due next phase
    for kind, it in phases:
        pool_now, pool_q = pool_q, []
        cur, ready = ready, None
        unit = cur[2] if cur else None
        if kind == "l1":
            if it >= 2:
                load(it, chunks=2)
            l1_agg(it, unit)
        else:
            l2_agg(it, unit)
        if cur is not None:
            cur[3]()
            if cur[0] == "l2":
                pool_q.append(cur[1])
        for p in pool_now:
            pool_block(p)
        if kind == "l1":
            ready = ("l1", it) + make_l1_dense(it)
        else:
            ready = ("l2", it) + make_l2_dense(it, fuse_pool=(it == items - 1))

    # tail: the last item's dense phase has no agg left to hide in
    kind, itl, unit, fin = ready
    for nb in range(NB):
        unit(nb)
    for p in pool_q:
        pool_block(p)       # fills the finish_stats latency with PE work
    fin()
    pool_block(itl)

    es.close()


_CACHE = {}


def _get_nc(items, general):
    key = (items, general)
    if key not in _CACHE:
        nc = bacc.Bacc("TRN2", target_bir_lowering=False, debug=False,
                       num_devices=N_CORES)
        with tile.TileContext(nc) as tc:
            io = _declare_io(nc, items, general)
            _build_core(nc, tc, io, items, general)
        nc.compile()
        _CACHE[key] = nc
    return _CACHE[key]


def make_in_maps(A_hat, X, W1, b1, g1, beta1, W2, b2, g2, beta2,
                 Wa, ba, Wl, bl):
    """Host-side prep: shard over batch, scale+fp8+transpose A, fold gammas."""
    B = A_hat.shape[0]
    items = B // N_CORES
    general = bool(np.any(beta1 != 0) or np.any(beta2 != 0)
                   or np.any(g1 <= 0) or np.any(g2 <= 0))
    if general:
        w2f = np.asarray(W2, np.float32)
        waf = np.asarray(Wa, np.float32)
        wlf = np.asarray(Wl, np.float32)
    else:
        w2f = np.asarray(g1, np.float32)[:, None] * W2
        waf = (np.asarray(g2, np.float32)[:, None] * Wa).astype(np.float32)
        wlf = (np.asarray(g2, np.float32)[:, None] * Wl).astype(np.float32)
    w1f = np.asarray(W1, np.float32)
    shared = {
        "w1": (w1f / ASCALE).astype(bf16),
        "w2": (w2f / ASCALE).astype(bf16),
        "b1bc": np.ascontiguousarray(
            np.broadcast_to(np.asarray(b1, np.float32), (P, H))),
        "b2bc": np.ascontiguousarray(
            np.broadcast_to(np.asarray(b2, np.float32), (P, H))),
        "wa": waf, "wl": wlf,
        "ba": np.asarray(ba, np.float32).reshape(K, 1).copy(),
        "bl": np.asarray(bl, np.float32).reshape(K, 1).copy(),
        "ones": np.ones((P, 1), bf16),
    }
    if general:
        for nm, v in (("g1bc", g1), ("be1bc", beta1),
                      ("g2bc", g2), ("be2bc", beta2)):
            shared[nm] = np.ascontiguousarray(
                np.broadcast_to(np.asarray(v, np.float32), (P, H)))
    in_maps = []
    for c in range(N_CORES):
        m = dict(shared)
        Ab = np.asarray(A_hat[c * items:(c + 1) * items], np.float32)
        Af = (Ab * np.float32(ASCALE)).astype(f8e4)
        # at4[it, c2, p, k, n] = A^T[(2*c2+k)*128+p, n] * ASCALE
        at = Af.transpose(0, 2, 1).reshape(items, NB2, 2, P, N)
        m["at4"] = np.ascontiguousarray(at.transpose(0, 1, 3, 2, 4))
        Xb = np.asarray(X[c * items:(c + 1) * items], np.float32).astype(f8e4)
        # x4[it, p, cb, f] = X[cb*128+p, f]
        m["x4"] = np.ascontiguousarray(
            Xb.reshape(items, NB, P, F).transpose(0, 2, 1, 3))
        in_maps.append(m)
    return in_maps, items, general


def kernel(**inputs):
    in_maps, items, general = make_in_maps(**inputs)
    nc = _get_nc(items, general)
    res = run_bass_kernel_spmd(nc, in_maps, core_ids=list(range(N_CORES)))
    pred = np.concatenate([res.results[c]["op"] for c in range(N_CORES)], 0)
    logits = np.concatenate([res.results[c]["ol"] for c in range(N_CORES)], 0)
    return (np.asarray(pred, np.float32), np.asarray(logits, np.float32))


# revision 42
# speedup vs baseline: 1.0848x; 1.0498x over previous
"""TRN2 Bass kernel for nn_GCNBasic (2-layer GCN, B=32, N=2048, F=128, H=256).

Sharding: data-parallel over batch B across 8 NeuronCores (4 items/core);
small weights replicated.  A_hat is scaled by 2^17, cast to fp8-e4m3 and
transposed on the HOST (quarters HBM traffic vs f32); X and H1 are also
fp8, so BOTH aggregation matmuls (88% of the FLOPs) run in DoubleRow fp8
perf mode: each PE instruction contracts TWO 128-deep k-planes
(stationary [128,2,128] fp8, moving [128,2,512] fp8) at 2x bf16 MACs
(measured 212 ns per [128,512]-out instruction, LDWEIGHTS fully
overlapped with the previous matmul's stream).  The 2^-17 unscale is
folded into W1/W2 (bf16 holds tiny values exactly), so all LayerNorm
math is numerically identical to a bf16 kernel.  W2 stays bf16: an fp8
W2's quantization error is constant across nodes and survives the
mean-pool readout (measured 2.6e-2 final error vs 4.0e-3 with W2 bf16).

  (AX)^T[f,n]  = sum_c2 X[pair]-stationary  @ A^T[pair]   (fp8 DoubleRow,
                                              4 psum 512-chunks live)
  H1pre[n,h]   = (AX)^T[:,nb]-stat. @ W1/2^17 (bf16)
  H1           = relu(LN(H1pre + b1)) -> fp8  (bias-add+rowsum in one
                 DVE scalar_tensor_tensor with accum_out — this is also
                 the PSUM drain, so PSUM recycles after ~420ns; sumsq in
                 a second DVE stt; relu apply on ACT from SBUF staging)
  (AH)^T[hh,n] = sum_c2 H1[pair]-stationary  @ A^T[pair]   (fp8 DoubleRow)
  H2pre[n,k]   = sum_hh (AH)^T[hh,nb]-stat.  @ diag(g1)W2/2^17
  H2           = relu(LN(H2pre + b2)) -> bf16
  g^T          = sum_nb H2[nb,kh]-stationary @ ones  (mean pool via PE)
  outputs      = diag(g2)Wa/Wl heads in fp32, biases added on ACT.

Items are software-pipelined at dense-matmul granularity: each dense
phase's 16 block-matmuls (+ their LN stats ops) drip two-per-c2 into
the NEXT aggregation phase's c2 loop; finish_stats + applies run in
quarters of 4 blocks so early applies overlap later stats; pool/head
blocks trail their LN2 applies by one phase; the last item fuses its
mean-pool accumulation into the apply quarters.  Scheduling findings
(measured): burst-dripping 4 units stalls the 2-deep ph rotation;
chunks>4 per A-tile DMA floods the SP sequencer (~565ns per issue);
3-item A prefetch and PSUM 5/3 rebalance were neutral-to-negative.

gamma folds (diag(g1)@W2, diag(g2)@Wa/Wl) are exact because relu(g*z)=
g*relu(z) for g>0; beta==0 fast path (the problem's setup_inputs always
produces gamma=1, beta=0); a general gamma/beta path exists as a fallback.

Known TRN2 pitfalls worked around here: tensor_tensor_reduce crashes the
device; ACT/DVE writes into PSUM are unstable (reads are fine); Pool
(gpsimd) has no PSUM access and no TensorScalarPtr opcode; PSUM-resident
pre-activations gate the in-order PE queue on the slow apply chain, so
pre-activations are drained to SBUF by the bias-add stt instead.
"""

from contextlib import ExitStack

import numpy as np
import ml_dtypes

import concourse.bacc as bacc
import concourse.mybir as mybir
import concourse.tile as tile
from concourse.bass_utils import run_bass_kernel_spmd

F32 = mybir.dt.float32
BF16 = mybir.dt.bfloat16
FP16 = mybir.dt.float16
F8 = mybir.dt.float8e4
bf16 = ml_dtypes.bfloat16
f8e4 = ml_dtypes.float8_e4m3

N = 2048
F = 128
H = 256
K = 64
P = 128
NB = N // P
NB2 = NB // 2
NCH = N // 512
EPS = 1e-5
N_CORES = 8
ASCALE = 2.0 ** 17  # A_hat -> e4m3 range; 1/ASCALE folded into W1/W2
DR = mybir.MatmulPerfMode.DoubleRow


def _declare_io(nc, items, general):
    io = {}
    io["at4"] = nc.dram_tensor("at4", [items, NB2, P, 2, N], F8,
                               kind="ExternalInput")
    io["x4"] = nc.dram_tensor("x4", [items, P, NB, F], F8,
                              kind="ExternalInput")
    io["w1"] = nc.dram_tensor("w1", [F, H], BF16, kind="ExternalInput")
    io["w2"] = nc.dram_tensor("w2", [H, H], BF16, kind="ExternalInput")
    io["b1bc"] = nc.dram_tensor("b1bc", [P, 2, H], F32, kind="ExternalInput")
    io["b2bc"] = nc.dram_tensor("b2bc", [P, 2, H], F32, kind="ExternalInput")
    io["wa"] = nc.dram_tensor("wa", [H, K], F32, kind="ExternalInput")
    io["wl"] = nc.dram_tensor("wl", [H, K], F32, kind="ExternalInput")
    io["ba"] = nc.dram_tensor("ba", [K, 1], F32, kind="ExternalInput")
    io["bl"] = nc.dram_tensor("bl", [K, 1], F32, kind="ExternalInput")
    io["ones"] = nc.dram_tensor("ones", [P, 1], BF16, kind="ExternalInput")
    if general:
        io["g1bc"] = nc.dram_tensor("g1bc", [P, H], F32, kind="ExternalInput")
        io["be1bc"] = nc.dram_tensor("be1bc", [P, H], F32,
                                     kind="ExternalInput")
        io["g2bc"] = nc.dram_tensor("g2bc", [P, H], F32, kind="ExternalInput")
        io["be2bc"] = nc.dram_tensor("be2bc", [P, H], F32,
                                     kind="ExternalInput")
    io["op"] = nc.dram_tensor("op", [items, K], F32, kind="ExternalOutput")
    io["ol"] = nc.dram_tensor("ol", [items, K], F32, kind="ExternalOutput")
    return io


def _build_core(nc, tc, io, items, general):
    at4, x4 = io["at4"], io["x4"]
    es = ExitStack()

    consts = es.enter_context(tc.tile_pool(name="consts", bufs=1))
    wts = es.enter_context(tc.tile_pool(name="wts", bufs=1))
    pool_at = es.enter_context(tc.tile_pool(name="at", bufs=2 * NB2))
    pool_xb = es.enter_context(tc.tile_pool(name="xb", bufs=2))
    pool_axT = es.enter_context(tc.tile_pool(name="axT", bufs=2))
    pool_h1 = es.enter_context(tc.tile_pool(name="h1", bufs=2))
    pool_ahT = es.enter_context(tc.tile_pool(name="ahT", bufs=2))
    pool_h2 = es.enter_context(tc.tile_pool(name="h2", bufs=1))
    pool_hc = es.enter_context(tc.tile_pool(name="hc", bufs=NB))
    pool_sq = es.enter_context(tc.tile_pool(name="sq", bufs=2))
    pool_st = es.enter_context(tc.tile_pool(name="st", bufs=4))
    pool_gsb = es.enter_context(tc.tile_pool(name="gsb", bufs=4))
    pool_osb = es.enter_context(tc.tile_pool(name="osb", bufs=4))

    ps_big = es.enter_context(tc.tile_pool(name="ps_big", bufs=6, space="PSUM"))
    ps_h = es.enter_context(tc.tile_pool(name="ps_h", bufs=2, space="PSUM"))
    ps_sm = ps_h  # pool/head tiles share the dense-phase banks

    eps_t = consts.tile([P, 1], F32)
    nc.vector.memset(eps_t[:], EPS)
    id1 = consts.tile([1, 1], FP16)
    nc.vector.memset(id1[:], 1.0)
    ones_b = consts.tile([P, 1], BF16)
    w1_t = wts.tile([P, H], BF16)
    w2_t = [wts.tile([P, H], BF16, tag=f"w2_{hh}", name=f"w2_{hh}")
            for hh in range(2)]
    b1_t = wts.tile([P, 2, H], F32)
    b2_t = wts.tile([P, 2, H], F32)
    wa_t = [wts.tile([P, K], F32, tag=f"wa_{hh}", name=f"wa_{hh}")
            for hh in range(2)]
    wl_t = [wts.tile([P, K], F32, tag=f"wl_{hh}", name=f"wl_{hh}")
            for hh in range(2)]
    ba_t = wts.tile([K, 1], F32)
    bl_t = wts.tile([K, 1], F32)
    gb_t = {}
    if general:
        for nm in ("g1bc", "be1bc", "g2bc", "be2bc"):
            gb_t[nm] = wts.tile([P, H], F32, tag=nm, name=nm)

    def emit_weight_dmas():
        nc.sync.dma_start(ones_b[:], io["ones"][:])
        nc.sync.dma_start(w1_t[:], io["w1"][:])
        for hh in range(2):
            nc.sync.dma_start(w2_t[hh][:], io["w2"][hh * P:(hh + 1) * P, :])
        nc.sync.dma_start(b1_t[:], io["b1bc"][:])
        nc.sync.dma_start(b2_t[:], io["b2bc"][:])
        for hh in range(2):
            nc.sync.dma_start(wa_t[hh][:], io["wa"][hh * P:(hh + 1) * P, :])
            nc.sync.dma_start(wl_t[hh][:], io["wl"][hh * P:(hh + 1) * P, :])
        nc.sync.dma_start(ba_t[:], io["ba"][:])
        nc.sync.dma_start(bl_t[:], io["bl"][:])
        for nm, t in gb_t.items():
            nc.sync.dma_start(t[:], io[nm][:])

    inv_h = 1.0 / H

    # per-item live tiles (indexed by item)
    at_t = [None] * items
    xb_t = [None] * items
    axT_t = [None] * items
    h1_t = [None] * items
    ahT_t = [None] * items
    h2_t = [None] * items
    st1_t = [None] * items
    st2_t = [None] * items

    def load(it, chunks=1):
        xb = pool_xb.tile([P, NB, F], F8, tag="xb", name=f"xb_{it}")
        nc.sync.dma_start(xb[:], x4[it])
        xb_t[it] = xb
        ats = [pool_at.tile([P, 2, N], F8, tag="at", name=f"at_{it}_{c2}")
               for c2 in range(NB2)]
        cw = N // chunks
        for c2 in range(NB2):
            for k in range(chunks):
                nc.sync.dma_start(
                    ats[c2][:, :, k * cw:(k + 1) * cw],
                    at4[it, c2, :, :, k * cw:(k + 1) * cw])
        at_t[it] = ats

    def copy_out(dst, src, who):
        # PSUM->SBUF copy-outs alternate ACT/DVE (Pool cannot read PSUM)
        if who % 2 == 0:
            nc.scalar.copy(dst, src)
        else:
            nc.vector.tensor_copy(dst, src)

    def l1_agg(it, unit=None):
        # two pending dense units (PE matmul + LN stats) dripped per c2
        # iteration so the LN chain paces alongside pure agg matmuls
        at, xb = at_t[it], xb_t[it]
        pb = [ps_big.tile([P, 512], F32, tag="big", name=f"ax_{it}_{j}")
              for j in range(NCH)]
        for c2 in range(NB2):
            for j in range(NCH):
                nc.tensor.matmul(pb[j][:], xb[:, 2 * c2:2 * c2 + 2, :],
                                 at[c2][:, :, j * 512:(j + 1) * 512],
                                 start=(c2 == 0), stop=(c2 == NB2 - 1),
                                 perf_mode=DR)
            if unit is not None:
                unit(c2)
        axT = pool_axT.tile([P, N], BF16, tag="axT", name=f"axT_{it}")
        for j in range(NCH):
            copy_out(axT[:, j * 512:(j + 1) * 512], pb[j][:], j)
        axT_t[it] = axT

    def pair_stats(q, ph2, b_t, bn6, hc2):
        # ONE 512-wide bias-add stt drains BOTH blocks of the PSUM bank
        # (the in-order PE queue is only gated ~690ns per pair), then a
        # bn_stats per block emits count/mean/count*var for even/odd
        # element halves -- no sumsq pass, no row-sum accumulation
        nc.vector.scalar_tensor_tensor(
            out=hc2[:], in0=ph2[:], scalar=1.0, in1=b_t[:],
            op0=mybir.AluOpType.mult, op1=mybir.AluOpType.add)
        for k in range(2):
            nc.vector.bn_stats(bn6[:, 2 * q + k, :], hc2[:, k, :])

    def finish_stats(bn6, st, lo=0, hi=NB):
        # exact recombination of bn_stats' even/odd halves on a [lo:hi)
        # nb slice: mu = (me+mo)/2, var = (cve+cvo)/H + ((me-mo)/2)^2
        b = bn6[:, lo:hi, :]
        s = st[:, :, lo:hi]
        AL = mybir.AluOpType
        nc.vector.tensor_tensor(out=s[:, 0, :], in0=b[:, :, 1],
                                in1=b[:, :, 4], op=AL.add)          # 2mu
        nc.vector.tensor_tensor(out=s[:, 1, :], in0=b[:, :, 1],
                                in1=b[:, :, 4], op=AL.subtract)     # d
        nc.vector.tensor_tensor(out=s[:, 2, :], in0=b[:, :, 2],
                                in1=b[:, :, 5], op=AL.add)          # cv
        nc.vector.tensor_scalar(out=s[:, 3, :], in0=s[:, 2, :],
                                scalar1=inv_h, scalar2=None,
                                op0=AL.mult)                        # cv/H
        nc.vector.scalar_tensor_tensor(
            out=s[:, 4, :], in0=s[:, 1, :], scalar=0.25, in1=s[:, 1, :],
            op0=AL.mult, op1=AL.mult)                               # d^2/4
        nc.vector.tensor_tensor(out=s[:, 3, :], in0=s[:, 3, :],
                                in1=s[:, 4, :], op=AL.add)          # var
        nc.scalar.activation(out=s[:, 5, :], in_=s[:, 3, :],
                             func=mybir.ActivationFunctionType.Sqrt,
                             bias=eps_t[:], scale=1.0)              # sd
        nc.vector.reciprocal(out=s[:, 6, :], in_=s[:, 5, :])        # 1/sd
        nc.vector.scalar_tensor_tensor(
            out=s[:, 7, :], in0=s[:, 0, :], scalar=-0.5, in1=s[:, 6, :],
            op0=AL.mult, op1=AL.mult)                               # -mu/sd

    def apply_ln(nb, hc, st, h_out, g_bc, be_bc):
        if not general:
            nc.scalar.activation(out=h_out, in_=hc[:],
                                 func=mybir.ActivationFunctionType.Relu,
                                 bias=st[:, 7, nb:nb + 1],
                                 scale=st[:, 6, nb:nb + 1])
        else:
            nc.scalar.activation(out=hc[:], in_=hc[:],
                                 func=mybir.ActivationFunctionType.Identity,
                                 bias=st[:, 7, nb:nb + 1],
                                 scale=st[:, 6, nb:nb + 1])
            nc.gpsimd.tensor_tensor(out=hc[:], in0=hc[:], in1=g_bc[:],
                                    op=mybir.AluOpType.mult)
            nc.vector.tensor_tensor(out=hc[:], in0=hc[:], in1=be_bc[:],
                                    op=mybir.AluOpType.add)
            nc.scalar.activation(out=h_out, in_=hc[:],
                                 func=mybir.ActivationFunctionType.Relu)

    def l2_agg(it, unit=None):
        at, h1 = at_t[it], h1_t[it]
        ahT = [pool_ahT.tile([P, N], BF16, tag="ahT", name=f"ahT_{it}_{hh}")
               for hh in range(2)]
        ucnt = 0
        for hh in range(2):
            pb = [ps_big.tile([P, 512], F32, tag="big",
                              name=f"ah_{it}_{hh}_{j}") for j in range(NCH)]
            for c2 in range(NB2):
                for j in range(NCH):
                    nc.tensor.matmul(
                        pb[j][:], h1[:, 2 * c2:2 * c2 + 2, hh * P:(hh + 1) * P],
                        at[c2][:, :, j * 512:(j + 1) * 512],
                        start=(c2 == 0), stop=(c2 == NB2 - 1), perf_mode=DR)
                if unit is not None:
                    if ucnt % 2 == 0:
                        unit(ucnt // 2)
                    ucnt += 1
            for j in range(NCH):
                copy_out(ahT[hh][:, j * 512:(j + 1) * 512], pb[j][:],
                         j + hh)
        ahT_t[it] = ahT

    pg_t = [None] * items

    def make_l1_dense(it):
        axT = axT_t[it]
        st1 = pool_st.tile([P, 8, NB], F32, tag="st", name=f"st1_{it}")
        bn1 = pool_st.tile([P, NB, 6], F32, tag="bn", name=f"bn1_{it}")
        h1 = pool_h1.tile([P, NB, H], F8, tag="h1", name=f"h1_{it}")
        hc1 = []
        st1_t[it], h1_t[it] = st1, h1

        def unit(q):
            ph2 = ps_h.tile([P, 2, H], F32, tag="h", name=f"p1_{it}_{q}")
            for k in range(2):
                nb = 2 * q + k
                nc.tensor.matmul(ph2[:, k, :], axT[:, nb * P:(nb + 1) * P],
                                 w1_t[:], start=True, stop=True,
                                 skip_group_check=True)
            hc2 = pool_hc.tile([P, 2, H], F32, tag="hc", name=f"hc1_{it}_{q}")
            pair_stats(q, ph2, b1_t, bn1, hc2)
            hc1.append(hc2)

        def fin():
            for q in range(0, NB, 4):
                finish_stats(bn1, st1, q, q + 4)
                for nb in range(q, q + 4):
                    apply_ln(nb, hc1[nb // 2][:, nb % 2, :], st1,
                             h1[:, nb, :], gb_t.get("g1bc"),
                             gb_t.get("be1bc"))

        return unit, fin

    def make_l2_dense(it, fuse_pool=False):
        ahT = ahT_t[it]
        st2 = pool_st.tile([P, 8, NB], F32, tag="st", name=f"st2_{it}")
        bn2 = pool_st.tile([P, NB, 6], F32, tag="bn", name=f"bn2_{it}")
        h2 = pool_h2.tile([P, NB, H], BF16, tag="h2", name=f"h2_{it}")
        hc2 = []
        st2_t[it], h2_t[it] = st2, h2

        def unit(q):
            ph2 = ps_h.tile([P, 2, H], F32, tag="h", name=f"p2_{it}_{q}")
            for k in range(2):
                nb = 2 * q + k
                for hh in range(2):
                    nc.tensor.matmul(ph2[:, k, :],
                                     ahT[hh][:, nb * P:(nb + 1) * P],
                                     w2_t[hh][:], start=(hh == 0),
                                     stop=(hh == 1), skip_group_check=True)
            hcp = pool_hc.tile([P, 2, H], F32, tag="hc", name=f"hc2_{it}_{q}")
            pair_stats(q, ph2, b2_t, bn2, hcp)
            hc2.append(hcp)

        def fin():
            if fuse_pool:
                # last item: mean-pool accumulation rides the apply
                # quarters so the tail chain stays short
                pg = [ps_sm.tile([P, 1], F32, tag="h", name=f"pg_{it}_{kh}")
                      for kh in range(2)]
                pg_t[it] = pg
            for q in range(0, NB, 4):
                finish_stats(bn2, st2, q, q + 4)
                for nb in range(q, q + 4):
                    apply_ln(nb, hc2[nb // 2][:, nb % 2, :], st2,
                             h2[:, nb, :], gb_t.get("g2bc"),
                             gb_t.get("be2bc"))
                    if fuse_pool:
                        for kh in range(2):
                            nc.tensor.matmul(pg[kh][:],
                                             h2[:, nb, kh * P:(kh + 1) * P],
                                             ones_b[:], start=(nb == 0),
                                             stop=(nb == NB - 1))

        return unit, fin

    def pool_block(it):
        h2 = h2_t[it]
        gsb = pool_gsb.tile([P, 2], F32, tag="g", name=f"g_{it}")
        if pg_t[it] is None:
            pg = [ps_sm.tile([P, 1], F32, tag="h", name=f"pg_{it}_{kh}")
                  for kh in range(2)]
            for nb in range(NB):
                for kh in range(2):
                    nc.tensor.matmul(pg[kh][:],
                                     h2[:, nb, kh * P:(kh + 1) * P],
                                     ones_b[:], start=(nb == 0),
                                     stop=(nb == NB - 1))
        else:
            pg = pg_t[it]
        for kh in range(2):
            nc.scalar.mul(gsb[:, kh:kh + 1], pg[kh][:], 1.0 / N)

        for hd, (w_t, b_t, out_d) in enumerate(
                ((wa_t, ba_t, io["op"]), (wl_t, bl_t, io["ol"]))):
            po = ps_sm.tile([K, 1], F32, tag="h", name=f"po_{it}_{hd}")
            for kh in range(2):
                nc.tensor.matmul(po[:], w_t[kh][:], gsb[:, kh:kh + 1],
                                 start=(kh == 0), stop=(kh == 1))
            osb = pool_osb.tile([K, 1], F32, tag="o", name=f"o_{it}_{hd}")
            nc.scalar.activation(out=osb[:], in_=po[:],
                                 func=mybir.ActivationFunctionType.Identity,
                                 bias=b_t[:], scale=1.0)
            nc.sync.dma_start(out_d[it:it + 1, :], osb[:])

    # ---- software pipeline: dense phases interleave into the next agg
    # phase's c2 loop; pool/head blocks trail their applies by one phase ----
    load(0, chunks=4)       # chunked so the first tiles land early
    emit_weight_dmas()
    if items == 1:
        l1_agg(0)
        unit, fin = make_l1_dense(0)
        for q in range(NB2):
            unit(q)
        fin()
        l2_agg(0)
        unit, fin = make_l2_dense(0, fuse_pool=True)
        for q in range(NB2):
            unit(q)
        fin()
        pool_block(0)
        es.close()
        return

    load(1, chunks=2)
    phases = [("l1", 0), ("l1", 1)]
    for it in range(items):
        phases.append(("l2", it))
        if it + 2 < items:
            phases.append(("l1", it + 2))

    ready = None            # (kind, it, unit, fin) pending dense phase
    pool_q = []             # items whose pool block is due next phase
    for kind, it in phases:
        pool_now, pool_q = pool_q, []
        cur, ready = ready, None
        unit = cur[2] if cur else None
        if kind == "l1":
            if it >= 2:
                load(it, chunks=2)
            l1_agg(it, unit)
        else:
            l2_agg(it, unit)
        if cur is not None:
            cur[3]()
            if cur[0] == "l2":
                pool_q.append(cur[1])
        for p in pool_now:
            pool_block(p)
        if kind == "l1":
            ready = ("l1", it) + make_l1_dense(it)
        else:
            ready = ("l2", it) + make_l2_dense(it, fuse_pool=(it == items - 1))

    # tail: the last item's dense phase has no agg left to hide in
    kind, itl, unit, fin = ready
    for q in range(NB2):
        unit(q)
    for p in pool_q:
        pool_block(p)       # fills the finish_stats latency with PE work
    fin()
    pool_block(itl)

    es.close()


_CACHE = {}


def _get_nc(items, general):
    key = (items, general)
    if key not in _CACHE:
        nc = bacc.Bacc("TRN2", target_bir_lowering=False, debug=False,
                       num_devices=N_CORES)
        with tile.TileContext(nc) as tc:
            io = _declare_io(nc, items, general)
            _build_core(nc, tc, io, items, general)
        nc.compile()
        _CACHE[key] = nc
    return _CACHE[key]


def make_in_maps(A_hat, X, W1, b1, g1, beta1, W2, b2, g2, beta2,
                 Wa, ba, Wl, bl):
    """Host-side prep: shard over batch, scale+fp8+transpose A, fold gammas."""
    B = A_hat.shape[0]
    items = B // N_CORES
    general = bool(np.any(beta1 != 0) or np.any(beta2 != 0)
                   or np.any(g1 <= 0) or np.any(g2 <= 0))
    if general:
        w2f = np.asarray(W2, np.float32)
        waf = np.asarray(Wa, np.float32)
        wlf = np.asarray(Wl, np.float32)
    else:
        w2f = np.asarray(g1, np.float32)[:, None] * W2
        waf = (np.asarray(g2, np.float32)[:, None] * Wa).astype(np.float32)
        wlf = (np.asarray(g2, np.float32)[:, None] * Wl).astype(np.float32)
    w1f = np.asarray(W1, np.float32)
    shared = {
        "w1": (w1f / ASCALE).astype(bf16),
        "w2": (w2f / ASCALE).astype(bf16),
        "b1bc": np.ascontiguousarray(
            np.broadcast_to(np.asarray(b1, np.float32), (P, 2, H))),
        "b2bc": np.ascontiguousarray(
            np.broadcast_to(np.asarray(b2, np.float32), (P, 2, H))),
        "wa": waf, "wl": wlf,
        "ba": np.asarray(ba, np.float32).reshape(K, 1).copy(),
        "bl": np.asarray(bl, np.float32).reshape(K, 1).copy(),
        "ones": np.ones((P, 1), bf16),
    }
    if general:
        for nm, v in (("g1bc", g1), ("be1bc", beta1),
                      ("g2bc", g2), ("be2bc", beta2)):
            shared[nm] = np.ascontiguousarray(
                np.broadcast_to(np.asarray(v, np.float32), (P, H)))
    in_maps = []
    for c in range(N_CORES):
        m = dict(shared)
        Ab = np.asarray(A_hat[c * items:(c + 1) * items], np.float32)
        Af = (Ab * np.float32(ASCALE)).astype(f8e4)
        # at4[it, c2, p, k, n] = A^T[(2*c2+k)*128+p, n] * ASCALE
        at = Af.transpose(0, 2, 1).reshape(items, NB2, 2, P, N)
        m["at4"] = np.ascontiguousarray(at.transpose(0, 1, 3, 2, 4))
        Xb = np.asarray(X[c * items:(c + 1) * items], np.float32).astype(f8e4)
        # x4[it, p, cb, f] = X[cb*128+p, f]
        m["x4"] = np.ascontiguousarray(
            Xb.reshape(items, NB, P, F).transpose(0, 2, 1, 3))
        in_maps.append(m)
    return in_maps, items, general


def kernel(**inputs):
    in_maps, items, general = make_in_maps(**inputs)
    nc = _get_nc(items, general)
    res = run_bass_kernel_spmd(nc, in_maps, core_ids=list(range(N_CORES)))
    pred = np.concatenate([res.results[c]["op"] for c in range(N_CORES)], 0)
    logits = np.concatenate([res.results[c]["ol"] for c in range(N_CORES)], 0)
    return (np.asarray(pred, np.float32), np.asarray(logits, np.float32))


# revision 43
# speedup vs baseline: 1.1616x; 1.0707x over previous
"""TRN2 Bass kernel for nn_GCNBasic (2-layer GCN, B=32, N=2048, F=128, H=256).

Sharding: data-parallel over batch B across 8 NeuronCores (4 items/core);
small weights replicated.  A_hat is scaled by 2^17, cast to fp8-e4m3 and
transposed on the HOST (quarters HBM traffic vs f32); X and H1 are also
fp8, so BOTH aggregation matmuls (88% of the FLOPs) run in DoubleRow fp8
perf mode: each PE instruction contracts TWO 128-deep k-planes
(stationary [128,2,128] fp8, moving [128,2,512] fp8) at 2x bf16 MACs
(measured 212 ns per [128,512]-out instruction, LDWEIGHTS fully
overlapped with the previous matmul's stream).  The 2^-17 unscale is
folded into W1/W2 (bf16 holds tiny values exactly), so all LayerNorm
math is numerically identical to a bf16 kernel.  W2 stays bf16: an fp8
W2's quantization error is constant across nodes and survives the
mean-pool readout (measured 2.6e-2 final error vs 4.0e-3 with W2 bf16).

  (AX)^T[f,n]  = sum_c2 X[pair]-stationary  @ A^T[pair]   (fp8 DoubleRow,
                                              4 psum 512-chunks live)
  H1pre[n,h]   = (AX)^T[:,nb]-stat. @ W1/2^17 (bf16)
  H1           = relu(LN(H1pre + b1)) -> fp8  (bias-add+rowsum in one
                 DVE scalar_tensor_tensor with accum_out — this is also
                 the PSUM drain, so PSUM recycles after ~420ns; sumsq in
                 a second DVE stt; relu apply on ACT from SBUF staging)
  (AH)^T[hh,n] = sum_c2 H1[pair]-stationary  @ A^T[pair]   (fp8 DoubleRow)
  H2pre[n,k]   = sum_hh (AH)^T[hh,nb]-stat.  @ diag(g1)W2/2^17
  H2           = relu(LN(H2pre + b2)) -> bf16
  g^T          = sum_nb H2[nb,kh]-stationary @ ones  (mean pool via PE)
  outputs      = diag(g2)Wa/Wl heads in fp32, biases added on ACT.

Items are software-pipelined at dense-matmul granularity: each dense
phase's 16 block-matmuls (+ their LN stats ops) drip two-per-c2 into
the NEXT aggregation phase's c2 loop; finish_stats + applies run in
quarters of 4 blocks so early applies overlap later stats; pool/head
blocks trail their LN2 applies by one phase; the last item fuses its
mean-pool accumulation into the apply quarters.  Scheduling findings
(measured): burst-dripping 4 units stalls the 2-deep ph rotation;
chunks>4 per A-tile DMA floods the SP sequencer (~565ns per issue);
3-item A prefetch and PSUM 5/3 rebalance were neutral-to-negative.

gamma folds (diag(g1)@W2, diag(g2)@Wa/Wl) are exact because relu(g*z)=
g*relu(z) for g>0; beta==0 fast path (the problem's setup_inputs always
produces gamma=1, beta=0); a general gamma/beta path exists as a fallback.

Known TRN2 pitfalls worked around here: tensor_tensor_reduce crashes the
device; ACT/DVE writes into PSUM are unstable (reads are fine); Pool
(gpsimd) has no PSUM access and no TensorScalarPtr opcode; PSUM-resident
pre-activations gate the in-order PE queue on the slow apply chain, so
pre-activations are drained to SBUF by the bias-add stt instead.
"""

from contextlib import ExitStack

import numpy as np
import ml_dtypes

import concourse.bacc as bacc
import concourse.mybir as mybir
import concourse.tile as tile
from concourse.bass_utils import run_bass_kernel_spmd

F32 = mybir.dt.float32
BF16 = mybir.dt.bfloat16
FP16 = mybir.dt.float16
F8 = mybir.dt.float8e4
bf16 = ml_dtypes.bfloat16
f8e4 = ml_dtypes.float8_e4m3

N = 2048
F = 128
H = 256
K = 64
P = 128
NB = N // P
NB2 = NB // 2
NCH = N // 512
EPS = 1e-5
N_CORES = 8
ASCALE = 2.0 ** 17  # A_hat -> e4m3 range; 1/ASCALE folded into W1/W2
DR = mybir.MatmulPerfMode.DoubleRow


def _declare_io(nc, items, general):
    io = {}
    io["at4"] = nc.dram_tensor("at4", [items, NB2, P, 2, N], F8,
                               kind="ExternalInput")
    io["x4"] = nc.dram_tensor("x4", [items, P, NB, F], F8,
                              kind="ExternalInput")
    io["w1"] = nc.dram_tensor("w1", [F, H], BF16, kind="ExternalInput")
    io["w2"] = nc.dram_tensor("w2", [H, H], BF16, kind="ExternalInput")
    io["b1bc"] = nc.dram_tensor("b1bc", [P, 2, H], F32, kind="ExternalInput")
    io["b2bc"] = nc.dram_tensor("b2bc", [P, 2, H], F32, kind="ExternalInput")
    io["wa"] = nc.dram_tensor("wa", [H, K], F32, kind="ExternalInput")
    io["wl"] = nc.dram_tensor("wl", [H, K], F32, kind="ExternalInput")
    io["ba"] = nc.dram_tensor("ba", [K, 1], F32, kind="ExternalInput")
    io["bl"] = nc.dram_tensor("bl", [K, 1], F32, kind="ExternalInput")
    io["ones"] = nc.dram_tensor("ones", [P, 1], BF16, kind="ExternalInput")
    if general:
        io["g1bc"] = nc.dram_tensor("g1bc", [P, H], F32, kind="ExternalInput")
        io["be1bc"] = nc.dram_tensor("be1bc", [P, H], F32,
                                     kind="ExternalInput")
        io["g2bc"] = nc.dram_tensor("g2bc", [P, H], F32, kind="ExternalInput")
        io["be2bc"] = nc.dram_tensor("be2bc", [P, H], F32,
                                     kind="ExternalInput")
    io["op"] = nc.dram_tensor("op", [items, K], F32, kind="ExternalOutput")
    io["ol"] = nc.dram_tensor("ol", [items, K], F32, kind="ExternalOutput")
    return io


def _build_core(nc, tc, io, items, general):
    at4, x4 = io["at4"], io["x4"]
    es = ExitStack()

    consts = es.enter_context(tc.tile_pool(name="consts", bufs=1))
    wts = es.enter_context(tc.tile_pool(name="wts", bufs=1))
    pool_at = es.enter_context(tc.tile_pool(name="at", bufs=2 * NB2))
    pool_xb = es.enter_context(tc.tile_pool(name="xb", bufs=2))
    pool_axT = es.enter_context(tc.tile_pool(name="axT", bufs=2))
    pool_h1 = es.enter_context(tc.tile_pool(name="h1", bufs=2))
    pool_ahT = es.enter_context(tc.tile_pool(name="ahT", bufs=2))
    pool_h2 = es.enter_context(tc.tile_pool(name="h2", bufs=1))
    pool_hc = es.enter_context(tc.tile_pool(name="hc", bufs=NB))
    pool_sq = es.enter_context(tc.tile_pool(name="sq", bufs=2))
    pool_st = es.enter_context(tc.tile_pool(name="st", bufs=4))
    pool_gsb = es.enter_context(tc.tile_pool(name="gsb", bufs=4))
    pool_osb = es.enter_context(tc.tile_pool(name="osb", bufs=4))

    ps_big = es.enter_context(tc.tile_pool(name="ps_big", bufs=6, space="PSUM"))
    ps_h = es.enter_context(tc.tile_pool(name="ps_h", bufs=2, space="PSUM"))
    ps_sm = ps_h  # pool/head tiles share the dense-phase banks

    eps_t = consts.tile([P, 1], F32)
    nc.vector.memset(eps_t[:], EPS)
    id1 = consts.tile([1, 1], FP16)
    nc.vector.memset(id1[:], 1.0)
    ones_b = consts.tile([P, 1], BF16)
    w1_t = wts.tile([P, H], BF16)
    w2_t = [wts.tile([P, H], BF16, tag=f"w2_{hh}", name=f"w2_{hh}")
            for hh in range(2)]
    b1_t = wts.tile([P, 2, H], F32)
    b2_t = wts.tile([P, 2, H], F32)
    wa_t = [wts.tile([P, K], F32, tag=f"wa_{hh}", name=f"wa_{hh}")
            for hh in range(2)]
    wl_t = [wts.tile([P, K], F32, tag=f"wl_{hh}", name=f"wl_{hh}")
            for hh in range(2)]
    ba_t = wts.tile([K, 1], F32)
    bl_t = wts.tile([K, 1], F32)
    gb_t = {}
    if general:
        for nm in ("g1bc", "be1bc", "g2bc", "be2bc"):
            gb_t[nm] = wts.tile([P, H], F32, tag=nm, name=nm)

    def emit_weight_dmas():
        nc.sync.dma_start(ones_b[:], io["ones"][:])
        nc.sync.dma_start(w1_t[:], io["w1"][:])
        for hh in range(2):
            nc.sync.dma_start(w2_t[hh][:], io["w2"][hh * P:(hh + 1) * P, :])
        nc.sync.dma_start(b1_t[:], io["b1bc"][:])
        nc.sync.dma_start(b2_t[:], io["b2bc"][:])
        for hh in range(2):
            nc.sync.dma_start(wa_t[hh][:], io["wa"][hh * P:(hh + 1) * P, :])
            nc.sync.dma_start(wl_t[hh][:], io["wl"][hh * P:(hh + 1) * P, :])
        nc.sync.dma_start(ba_t[:], io["ba"][:])
        nc.sync.dma_start(bl_t[:], io["bl"][:])
        for nm, t in gb_t.items():
            nc.sync.dma_start(t[:], io[nm][:])

    inv_h = 1.0 / H

    # per-item live tiles (indexed by item)
    at_t = [None] * items
    xb_t = [None] * items
    axT_t = [None] * items
    h1_t = [None] * items
    ahT_t = [None] * items
    h2_t = [None] * items
    st1_t = [None] * items
    st2_t = [None] * items

    def load(it, chunks=1):
        xb = pool_xb.tile([P, NB, F], F8, tag="xb", name=f"xb_{it}")
        nc.sync.dma_start(xb[:], x4[it])
        xb_t[it] = xb
        ats = [pool_at.tile([P, 2, N], F8, tag="at", name=f"at_{it}_{c2}")
               for c2 in range(NB2)]
        cw = N // chunks
        for c2 in range(NB2):
            for k in range(chunks):
                nc.sync.dma_start(
                    ats[c2][:, :, k * cw:(k + 1) * cw],
                    at4[it, c2, :, :, k * cw:(k + 1) * cw])
        at_t[it] = ats

    def copy_out(dst, src, who):
        # PSUM->SBUF copy-outs alternate ACT/DVE (Pool cannot read PSUM)
        if who % 2 == 0:
            nc.scalar.copy(dst, src)
        else:
            nc.vector.tensor_copy(dst, src)

    def l1_agg(it, unit=None):
        # two pending dense units (PE matmul + LN stats) dripped per c2
        # iteration so the LN chain paces alongside pure agg matmuls
        at, xb = at_t[it], xb_t[it]
        pb = [ps_big.tile([P, 512], F32, tag="big", name=f"ax_{it}_{j}")
              for j in range(NCH)]
        for c2 in range(NB2):
            for j in range(NCH):
                nc.tensor.matmul(pb[j][:], xb[:, 2 * c2:2 * c2 + 2, :],
                                 at[c2][:, :, j * 512:(j + 1) * 512],
                                 start=(c2 == 0), stop=(c2 == NB2 - 1),
                                 perf_mode=DR)
            if unit is not None:
                unit(c2)
        axT = pool_axT.tile([P, N], BF16, tag="axT", name=f"axT_{it}")
        for j in range(NCH):
            copy_out(axT[:, j * 512:(j + 1) * 512], pb[j][:], 0)
        axT_t[it] = axT

    def pair_stats(q, ph2, b_t, bn6, hc2):
        # ONE 512-wide bias-add stt drains BOTH blocks of the PSUM bank
        # (the in-order PE queue is only gated ~690ns per pair), then a
        # bn_stats per block emits count/mean/count*var for even/odd
        # element halves -- no sumsq pass, no row-sum accumulation
        nc.vector.scalar_tensor_tensor(
            out=hc2[:], in0=ph2[:], scalar=1.0, in1=b_t[:],
            op0=mybir.AluOpType.mult, op1=mybir.AluOpType.add)
        for k in range(2):
            nc.vector.bn_stats(bn6[:, 2 * q + k, :], hc2[:, k, :])

    def finish_stats(bn6, st, lo=0, hi=NB):
        # exact recombination of bn_stats' even/odd halves on a [lo:hi)
        # nb slice: mu = (me+mo)/2, var = (cve+cvo)/H + ((me-mo)/2)^2
        b = bn6[:, lo:hi, :]
        s = st[:, :, lo:hi]
        AL = mybir.AluOpType
        nc.vector.tensor_tensor(out=s[:, 0, :], in0=b[:, :, 1],
                                in1=b[:, :, 4], op=AL.add)          # 2mu
        nc.vector.tensor_tensor(out=s[:, 1, :], in0=b[:, :, 1],
                                in1=b[:, :, 4], op=AL.subtract)     # d
        nc.vector.tensor_tensor(out=s[:, 2, :], in0=b[:, :, 2],
                                in1=b[:, :, 5], op=AL.add)          # cv
        nc.vector.tensor_scalar(out=s[:, 3, :], in0=s[:, 2, :],
                                scalar1=inv_h, scalar2=None,
                                op0=AL.mult)                        # cv/H
        nc.vector.scalar_tensor_tensor(
            out=s[:, 4, :], in0=s[:, 1, :], scalar=0.25, in1=s[:, 1, :],
            op0=AL.mult, op1=AL.mult)                               # d^2/4
        nc.vector.tensor_tensor(out=s[:, 3, :], in0=s[:, 3, :],
                                in1=s[:, 4, :], op=AL.add)          # var
        nc.scalar.activation(out=s[:, 5, :], in_=s[:, 3, :],
                             func=mybir.ActivationFunctionType.Sqrt,
                             bias=eps_t[:], scale=1.0)              # sd
        nc.vector.reciprocal(out=s[:, 6, :], in_=s[:, 5, :])        # 1/sd
        nc.vector.scalar_tensor_tensor(
            out=s[:, 7, :], in0=s[:, 0, :], scalar=-0.5, in1=s[:, 6, :],
            op0=AL.mult, op1=AL.mult)                               # -mu/sd

    def apply_ln(nb, hc, st, h_out, g_bc, be_bc):
        if not general:
            nc.scalar.activation(out=h_out, in_=hc[:],
                                 func=mybir.ActivationFunctionType.Relu,
                                 bias=st[:, 7, nb:nb + 1],
                                 scale=st[:, 6, nb:nb + 1])
        else:
            nc.scalar.activation(out=hc[:], in_=hc[:],
                                 func=mybir.ActivationFunctionType.Identity,
                                 bias=st[:, 7, nb:nb + 1],
                                 scale=st[:, 6, nb:nb + 1])
            nc.gpsimd.tensor_tensor(out=hc[:], in0=hc[:], in1=g_bc[:],
                                    op=mybir.AluOpType.mult)
            nc.vector.tensor_tensor(out=hc[:], in0=hc[:], in1=be_bc[:],
                                    op=mybir.AluOpType.add)
            nc.scalar.activation(out=h_out, in_=hc[:],
                                 func=mybir.ActivationFunctionType.Relu)

    def l2_agg(it, unit=None):
        at, h1 = at_t[it], h1_t[it]
        ahT = [pool_ahT.tile([P, N], BF16, tag="ahT", name=f"ahT_{it}_{hh}")
               for hh in range(2)]
        ucnt = 0
        for hh in range(2):
            pb = [ps_big.tile([P, 512], F32, tag="big",
                              name=f"ah_{it}_{hh}_{j}") for j in range(NCH)]
            for c2 in range(NB2):
                for j in range(NCH):
                    nc.tensor.matmul(
                        pb[j][:], h1[:, 2 * c2:2 * c2 + 2, hh * P:(hh + 1) * P],
                        at[c2][:, :, j * 512:(j + 1) * 512],
                        start=(c2 == 0), stop=(c2 == NB2 - 1), perf_mode=DR)
                if unit is not None:
                    if ucnt % 2 == 0:
                        unit(ucnt // 2)
                    ucnt += 1
            for j in range(NCH):
                copy_out(ahT[hh][:, j * 512:(j + 1) * 512], pb[j][:],
                         j + hh)
        ahT_t[it] = ahT

    pg_t = [None] * items

    def make_l1_dense(it):
        axT = axT_t[it]
        st1 = pool_st.tile([P, 8, NB], F32, tag="st", name=f"st1_{it}")
        bn1 = pool_st.tile([P, NB, 6], F32, tag="bn", name=f"bn1_{it}")
        h1 = pool_h1.tile([P, NB, H], F8, tag="h1", name=f"h1_{it}")
        hc1 = []
        st1_t[it], h1_t[it] = st1, h1

        def unit(q):
            ph2 = ps_h.tile([P, 2, H], F32, tag="h", name=f"p1_{it}_{q}")
            for k in range(2):
                nb = 2 * q + k
                nc.tensor.matmul(ph2[:, k, :], axT[:, nb * P:(nb + 1) * P],
                                 w1_t[:], start=True, stop=True,
                                 skip_group_check=True)
            hc2 = pool_hc.tile([P, 2, H], F32, tag="hc", name=f"hc1_{it}_{q}")
            pair_stats(q, ph2, b1_t, bn1, hc2)
            hc1.append(hc2)

        def fin():
            for q in range(0, NB, 4):
                finish_stats(bn1, st1, q, q + 4)
                for nb in range(q, q + 4):
                    apply_ln(nb, hc1[nb // 2][:, nb % 2, :], st1,
                             h1[:, nb, :], gb_t.get("g1bc"),
                             gb_t.get("be1bc"))

        return unit, fin

    def make_l2_dense(it, fuse_pool=False):
        ahT = ahT_t[it]
        st2 = pool_st.tile([P, 8, NB], F32, tag="st", name=f"st2_{it}")
        bn2 = pool_st.tile([P, NB, 6], F32, tag="bn", name=f"bn2_{it}")
        h2 = pool_h2.tile([P, NB, H], BF16, tag="h2", name=f"h2_{it}")
        hc2 = []
        st2_t[it], h2_t[it] = st2, h2

        def unit(q):
            ph2 = ps_h.tile([P, 2, H], F32, tag="h", name=f"p2_{it}_{q}")
            for k in range(2):
                nb = 2 * q + k
                for hh in range(2):
                    nc.tensor.matmul(ph2[:, k, :],
                                     ahT[hh][:, nb * P:(nb + 1) * P],
                                     w2_t[hh][:], start=(hh == 0),
                                     stop=(hh == 1), skip_group_check=True)
            hcp = pool_hc.tile([P, 2, H], F32, tag="hc", name=f"hc2_{it}_{q}")
            pair_stats(q, ph2, b2_t, bn2, hcp)
            hc2.append(hcp)

        def fin():
            if fuse_pool:
                # last item: mean-pool accumulation rides the apply
                # quarters so the tail chain stays short
                pg = [ps_sm.tile([P, 1], F32, tag="h", name=f"pg_{it}_{kh}")
                      for kh in range(2)]
                pg_t[it] = pg
            for q in range(0, NB, 4):
                finish_stats(bn2, st2, q, q + 4)
                for nb in range(q, q + 4):
                    apply_ln(nb, hc2[nb // 2][:, nb % 2, :], st2,
                             h2[:, nb, :], gb_t.get("g2bc"),
                             gb_t.get("be2bc"))
                    if fuse_pool:
                        for kh in range(2):
                            nc.tensor.matmul(pg[kh][:],
                                             h2[:, nb, kh * P:(kh + 1) * P],
                                             ones_b[:], start=(nb == 0),
                                             stop=(nb == NB - 1))

        return unit, fin

    def pool_block(it):
        h2 = h2_t[it]
        gsb = pool_gsb.tile([P, 2], F32, tag="g", name=f"g_{it}")
        if pg_t[it] is None:
            pg = [ps_sm.tile([P, 1], F32, tag="h", name=f"pg_{it}_{kh}")
                  for kh in range(2)]
            for nb in range(NB):
                for kh in range(2):
                    nc.tensor.matmul(pg[kh][:],
                                     h2[:, nb, kh * P:(kh + 1) * P],
                                     ones_b[:], start=(nb == 0),
                                     stop=(nb == NB - 1))
        else:
            pg = pg_t[it]
        for kh in range(2):
            nc.scalar.mul(gsb[:, kh:kh + 1], pg[kh][:], 1.0 / N)

        for hd, (w_t, b_t, out_d) in enumerate(
                ((wa_t, ba_t, io["op"]), (wl_t, bl_t, io["ol"]))):
            po = ps_sm.tile([K, 1], F32, tag="h", name=f"po_{it}_{hd}")
            for kh in range(2):
                nc.tensor.matmul(po[:], w_t[kh][:], gsb[:, kh:kh + 1],
                                 start=(kh == 0), stop=(kh == 1))
            osb = pool_osb.tile([K, 1], F32, tag="o", name=f"o_{it}_{hd}")
            nc.scalar.activation(out=osb[:], in_=po[:],
                                 func=mybir.ActivationFunctionType.Identity,
                                 bias=b_t[:], scale=1.0)
            nc.sync.dma_start(out_d[it:it + 1, :], osb[:])

    # ---- software pipeline: dense phases interleave into the next agg
    # phase's c2 loop; pool/head blocks trail their applies by one phase ----
    load(0, chunks=2)       # chunked so the first tiles land early
    emit_weight_dmas()
    if items == 1:
        l1_agg(0)
        unit, fin = make_l1_dense(0)
        for q in range(NB2):
            unit(q)
        fin()
        l2_agg(0)
        unit, fin = make_l2_dense(0, fuse_pool=True)
        for q in range(NB2):
            unit(q)
        fin()
        pool_block(0)
        es.close()
        return

    load(1, chunks=2)
    phases = [("l1", 0), ("l1", 1)]
    for it in range(items):
        phases.append(("l2", it))
        if it + 2 < items:
            phases.append(("l1", it + 2))

    ready = None            # (kind, it, unit, fin) pending dense phase
    pool_q = []             # items whose pool block is due next phase
    for kind, it in phases:
        pool_now, pool_q = pool_q, []
        cur, ready = ready, None
        unit = cur[2] if cur else None
        if kind == "l1":
            if it >= 2:
                load(it, chunks=2)
            l1_agg(it, unit)
        else:
            l2_agg(it, unit)
        if cur is not None:
            cur[3]()
            if cur[0] == "l2":
                pool_q.append(cur[1])
        for p in pool_now:
            pool_block(p)
        if kind == "l1":
            ready = ("l1", it) + make_l1_dense(it)
        else:
            ready = ("l2", it) + make_l2_dense(it, fuse_pool=(it == items - 1))

    # tail: the last item's dense phase has no agg left to hide in
    kind, itl, unit, fin = ready
    for q in range(NB2):
        unit(q)
    for p in pool_q:
        pool_block(p)       # fills the finish_stats latency with PE work
    fin()
    pool_block(itl)

    es.close()


_CACHE = {}


def _get_nc(items, general):
    key = (items, general)
    if key not in _CACHE:
        nc = bacc.Bacc("TRN2", target_bir_lowering=False, debug=False,
                       num_devices=N_CORES)
        with tile.TileContext(nc) as tc:
            io = _declare_io(nc, items, general)
            _build_core(nc, tc, io, items, general)
        nc.compile()
        _CACHE[key] = nc
    return _CACHE[key]


def make_in_maps(A_hat, X, W1, b1, g1, beta1, W2, b2, g2, beta2,
                 Wa, ba, Wl, bl):
    """Host-side prep: shard over batch, scale+fp8+transpose A, fold gammas."""
    B = A_hat.shape[0]
    items = B // N_CORES
    general = bool(np.any(beta1 != 0) or np.any(beta2 != 0)
                   or np.any(g1 <= 0) or np.any(g2 <= 0))
    if general:
        w2f = np.asarray(W2, np.float32)
        waf = np.asarray(Wa, np.float32)
        wlf = np.asarray(Wl, np.float32)
    else:
        w2f = np.asarray(g1, np.float32)[:, None] * W2
        waf = (np.asarray(g2, np.float32)[:, None] * Wa).astype(np.float32)
        wlf = (np.asarray(g2, np.float32)[:, None] * Wl).astype(np.float32)
    w1f = np.asarray(W1, np.float32)
    shared = {
        "w1": (w1f / ASCALE).astype(bf16),
        "w2": (w2f / ASCALE).astype(bf16),
        "b1bc": np.ascontiguousarray(
            np.broadcast_to(np.asarray(b1, np.float32), (P, 2, H))),
        "b2bc": np.ascontiguousarray(
            np.broadcast_to(np.asarray(b2, np.float32), (P, 2, H))),
        "wa": waf, "wl": wlf,
        "ba": np.asarray(ba, np.float32).reshape(K, 1).copy(),
        "bl": np.asarray(bl, np.float32).reshape(K, 1).copy(),
        "ones": np.ones((P, 1), bf16),
    }
    if general:
        for nm, v in (("g1bc", g1), ("be1bc", beta1),
                      ("g2bc", g2), ("be2bc", beta2)):
            shared[nm] = np.ascontiguousarray(
                np.broadcast_to(np.asarray(v, np.float32), (P, H)))
    in_maps = []
    for c in range(N_CORES):
        m = dict(shared)
        Ab = np.asarray(A_hat[c * items:(c + 1) * items], np.float32)
        Af = (Ab * np.float32(ASCALE)).astype(f8e4)
        # at4[it, c2, p, k, n] = A^T[(2*c2+k)*128+p, n] * ASCALE
        at = Af.transpose(0, 2, 1).reshape(items, NB2, 2, P, N)
        m["at4"] = np.ascontiguousarray(at.transpose(0, 1, 3, 2, 4))
        Xb = np.asarray(X[c * items:(c + 1) * items], np.float32).astype(f8e4)
        # x4[it, p, cb, f] = X[cb*128+p, f]
        m["x4"] = np.ascontiguousarray(
            Xb.reshape(items, NB, P, F).transpose(0, 2, 1, 3))
        in_maps.append(m)
    return in_maps, items, general


def kernel(**inputs):
    in_maps, items, general = make_in_maps(**inputs)
    nc = _get_nc(items, general)
    res = run_bass_kernel_spmd(nc, in_maps, core_ids=list(range(N_CORES)))
    pred = np.concatenate([res.results[c]["op"] for c in range(N_CORES)], 0)
    logits = np.concatenate([res.results[c]["ol"] for c in range(N_CORES)], 0)
    return (np.asarray(pred, np.float32), np.asarray(logits, np.float32))
